# revision 39
# baseline (speedup 1.0000x reference)
"""Trainium2 Bass kernel for nn_ContinuousConvolutionBlock (gnn_message_passing).

Strategy (per sharding hint: partition points across 8 cores; each core owns its
queries' scatter-reduce and tap-GEMM; filter + dense weights replicated):

Host side (index plumbing / input marshalling only — zero FLOPs):
  - qry_idx is sorted; queries are grouped into 8-query blocks, blocks paired
    into 128-edge-slot "chunks" (two-pointer bin packing, ~3% padding).
  - Consecutive block ranges are assigned to the 8 cores; per-core per-slot
    payload arrays (pos[src]/pos[qry] coord-major, feats[src] bf16, int16
    scatter indices qloc*16+t, local query ids) are marshalled and DMA'd.

Device side (all FLOP-bearing compute), spread over all five engines:
  - Geometry (DVE+ACT): ball->cube volume-preserving map on UNSCALED deltas
    (map is linear in scale; 2/EXTENT folds into the final grid transform),
    computed in two W-slabs so the group pipeline starts after slab 0;
    4-wide trilinear corner weights via the hat identity
    w4[j] = relu(1 - |g - j|).
  - L build: L[slot, (q16, az, ay)] = one-hot scatter of zy = w4z (x) w4y
    into the query slot — 3 of 4 quarters via gpsimd local_scatter with
    host-precomputed indices (zeroes dst, skips -1 padding), 1 quarter via
    DVE broadcast-mult with a query one-hot (engine balance).
  - R build (DVE): R[slot, (ax,c)] = w4x (x) feats, bf16, 16 chunks/op.
  - Stage-1 (PE): A^T[(ax,c), (q,az,ay)] = R^T @ L per chunk, bf16,
    fp32 PSUM accumulate; PSUM staged to SBUF as bf16 on ACT (3) + DVE (1).
  - Tap-GEMM (PE): out^T += G_t^T @ A^T-slices over 16 taps, bf16, fused
    over group PAIRS (512-wide moving dim) and software-pipelined one pair
    behind stage-1 so the PE stream stays dense.
  - Dense branch (PE, bf16) issued first so PE warms during the prologue.
  Outputs are produced transposed ([64, nq]); host transposes/reorders back.
"""
import sys
import os
sys.path.insert(0, '/opt/trn_rl_repo')
import numpy as np
import ml_dtypes

BF16 = ml_dtypes.bfloat16

N = 30000
CIN = 32
COUT = 64
KS = 4
EXTENT = 0.08
NCORES = 8
NBLK = N // 8  # 3750 eight-query blocks

_COMPILED = {}


# ----------------------------------------------------------------------------
# Host planning
# ----------------------------------------------------------------------------
def _plan(qry_idx):
    deg = np.bincount(qry_idx, minlength=N)
    bsz = deg.reshape(NBLK, 8).sum(1)
    bstart = np.concatenate([[0], np.cumsum(bsz)]).astype(np.int64)
    per = [NBLK // NCORES + (1 if c < NBLK % NCORES else 0) for c in range(NCORES)]
    b0 = np.concatenate([[0], np.cumsum(per)]).astype(np.int64)
    plans = []
    for c in range(NCORES):
        blocks = list(range(b0[c], b0[c + 1]))
        asc = sorted(blocks, key=lambda b: bsz[b])
        chunks = []
        lo, hi = 0, len(asc) - 1
        while lo <= hi:
            if lo == hi:
                chunks.append((asc[hi], None)); break
            if bsz[asc[hi]] + bsz[asc[lo]] <= 128:
                chunks.append((asc[hi], asc[lo])); hi -= 1; lo += 1
            else:
                chunks.append((asc[hi], None)); hi -= 1
        plans.append(dict(blocks=blocks, chunks=chunks, q0=int(8 * b0[c]),
                          nq=int(8 * (b0[c + 1] - b0[c]))))
    return plans, bstart, bsz


def _pack_core(plan_c, bstart, pos, feats_bf, qry_idx, src_idx, NCHP):
    """Per-slot payload: pos coord-major, feats bf16, int16 scatter indices."""
    possrc = np.zeros((128, 4, NCHP), np.float32)
    posqry = np.zeros((128, 4, NCHP), np.float32)
    fsrc = np.zeros((128, NCHP, CIN), BF16)
    qloc = np.full((128, NCHP), -1, np.int32)
    for ci, (bA, bB) in enumerate(plan_c['chunks']):
        s = 0
        for half, b in enumerate((bA, bB)):
            if b is None:
                continue
            e0, e1 = int(bstart[b]), int(bstart[b + 1])
            n = e1 - e0
            sl = slice(s, s + n)
            possrc[sl, 0:3, ci] = pos[src_idx[e0:e1]]
            posqry[sl, 0:3, ci] = pos[qry_idx[e0:e1]]
            fsrc[sl, ci, :] = feats_bf[src_idx[e0:e1]]
            qloc[sl, ci] = (qry_idx[e0:e1] - 8 * b) + 8 * half
            s += n
    # scatter index: within each 4-chunk scatter window,
    # idx[slot, ci, t] = (ci%4)*256 + qloc*16 + t  (or -1 for padding)
    t16 = np.arange(16, dtype=np.int32)
    idx = ((np.arange(NCHP, dtype=np.int32)[None, :] % 4) * 256
           + qloc * 16)[:, :, None] + t16[None, None, :]
    idx = np.where((qloc < 0)[:, :, None], -1, idx).astype(np.int16)
    return possrc, posqry, fsrc, idx, qloc.astype(np.float32)


# ----------------------------------------------------------------------------
# Device kernel
# ----------------------------------------------------------------------------
def _build_bass(NCHP, NQ):
    import concourse.bass as bass
    import concourse.tile as tile
    from concourse import bacc, mybir
    from concourse.bass import AP

    f32 = mybir.dt.float32
    f32r = mybir.dt.float32r
    bf16 = mybir.dt.bfloat16
    i32 = mybir.dt.int32
    i16 = mybir.dt.int16
    ALU = mybir.AluOpType
    ACT = mybir.ActivationFunctionType
    EPS = 1e-12
    F4PI = float(4.0 / np.pi)
    SC = 1.5 * (2.0 / EXTENT)  # grid scale folded with coord normalization

    nc = bacc.Bacc("TRN2", target_bir_lowering=False, debug=False)

    W = NCHP
    NGRP = W // 16
    # uneven slab splits (at group boundaries): a tiny first slab so the
    # scatter/matmul pipeline starts early; later slabs' geometry is fed
    # incrementally into DVE slack during earlier groups
    gb = sorted(set(min(x, NGRP) for x in (0, 8, NGRP)))
    SLABS = [(gb[i] * 16, (gb[i + 1] - gb[i]) * 16)
             for i in range(len(gb) - 1) if gb[i + 1] > gb[i]]
    SLAB_OF = {}
    for si, (o0, Ws) in enumerate(SLABS):
        for g in range(o0 // 16, (o0 + Ws) // 16):
            SLAB_OF[g] = si

    possrc = nc.dram_tensor("possrc", (128, 4, W), f32, kind="ExternalInput")
    posqry = nc.dram_tensor("posqry", (128, 4, W), f32, kind="ExternalInput")
    fsrc = nc.dram_tensor("fsrc", (128, W, CIN), bf16, kind="ExternalInput")
    sidx = nc.dram_tensor("sidx", (128, W, 16), i16, kind="ExternalInput")
    qlocf = nc.dram_tensor("qlocf", (128, W), f32, kind="ExternalInput")
    g2 = nc.dram_tensor("g2", (128, 16 * 64), f32, kind="ExternalInput")
    featsT = nc.dram_tensor("featsT", (CIN, NQ), bf16, kind="ExternalInput")
    denw = nc.dram_tensor("denw", (CIN, COUT), bf16, kind="ExternalInput")
    denb = nc.dram_tensor("denb", (COUT, 1), f32, kind="ExternalInput")

    outconvT = nc.dram_tensor("outconvT", (COUT, NQ), f32, kind="ExternalOutput")
    outdenseT = nc.dram_tensor("outdenseT", (COUT, NQ), f32, kind="ExternalOutput")

    with tile.TileContext(nc) as tc:
        with tc.tile_pool(name="inp", bufs=1) as inp, \
             tc.tile_pool(name="geo", bufs=1) as geo, \
             tc.tile_pool(name="w4p", bufs=2) as w4p, \
             tc.tile_pool(name="tmp", bufs=1) as tmp, \
             tc.tile_pool(name="lp", bufs=3) as lpool, \
             tc.tile_pool(name="rp", bufs=3) as rpool, \
             tc.tile_pool(name="at", bufs=2) as atp, \
             tc.tile_pool(name="outp", bufs=2) as outp, \
             tc.tile_pool(name="ps1", bufs=3, space="PSUM") as ps1, \
             tc.tile_pool(name="ps2", bufs=1, space="PSUM") as ps2, \
             tc.tile_pool(name="ps3", bufs=1, space="PSUM") as ps3:

            # ---------------- input DMAs ----------------
            t_ps = inp.tile([128, 4, W], f32)
            t_pq = inp.tile([128, 4, W], f32)
            t_f = inp.tile([128, W, CIN], bf16)
            t_si = inp.tile([128, W, 16], i16)
            t_ql = inp.tile([128, W], f32)
            t_g2 = inp.tile([128, 16 * 64], f32)
            t_ftT = inp.tile([CIN, NQ], bf16)
            t_dw = inp.tile([CIN, COUT], bf16)
            t_db = inp.tile([COUT, 1], f32)
            # slab-0 pos first so geometry can start early
            W0 = SLABS[0][1]
            nc.sync.dma_start(t_ps[:, :, 0:W0], possrc[:, :, 0:W0])
            nc.sync.dma_start(t_pq[:, :, 0:W0], posqry[:, :, 0:W0])
            nc.sync.dma_start(t_si[:], sidx[:])
            nc.sync.dma_start(t_f[:], fsrc[:])
            nc.sync.dma_start(t_dw[:], denw[:])
            nc.sync.dma_start(t_db[:], denb[:])
            nc.sync.dma_start(t_ps[:, :, W0:W], possrc[:, :, W0:W])
            nc.sync.dma_start(t_pq[:, :, W0:W], posqry[:, :, W0:W])
            nc.sync.dma_start(t_ftT[:], featsT[:])
            nc.sync.dma_start(t_g2[:], g2[:])
            nc.sync.dma_start(t_ql[:], qlocf[:])

            # iota constants: io4m = j - 1.5 (j=0..3), io16 = 0..15
            io4i = tmp.tile([128, 4], i32)
            nc.gpsimd.iota(io4i[:], pattern=[[1, 4]], base=0, channel_multiplier=0)
            io4m = tmp.tile([128, 4], f32)
            nc.scalar.activation(io4m[:], io4i[:], ACT.Copy, bias=-1.5)
            t_g2r = inp.tile([128, 16 * 64], f32r)
            nc.vector.tensor_copy(t_g2r[:], t_g2[:])
            io16i = tmp.tile([128, 16], i32)
            nc.gpsimd.iota(io16i[:], pattern=[[1, 16]], base=0,
                           channel_multiplier=0)
            io16 = tmp.tile([128, 16], f32)
            nc.scalar.activation(io16[:], io16i[:], ACT.Copy)

            TT = nc.vector.tensor_tensor
            TS = nc.vector.tensor_scalar
            STT = nc.vector.scalar_tensor_tensor
            AA = nc.scalar.activation

            def fl(t, n, off=0):  # flat [128, n] view
                return AP(t.tensor, t[:].offset + off, [t[:].ap[0], [1, n]])

            def sl(t, off, n, *dims):  # strided view: dims = (stride, count)*
                pat = [t[:].ap[0]] + [[s, c] for (s, c) in dims] if dims else \
                      [t[:].ap[0], [1, n]]
                return AP(t.tensor, t[:].offset + off, pat)

            # ------------- dense branch, issued one segment per group ------
            def dense_seg(s):
                j0 = s * 512
                j1 = min(NQ, j0 + 512)
                pd = ps3.tile([COUT, 512], f32, space="PSUM", tag="den")
                nc.tensor.matmul(
                    out=pd[:, 0:j1 - j0],
                    lhsT=t_dw[:],
                    rhs=t_ftT[:, j0:j1],
                    start=True, stop=True)
                odt = outp.tile([COUT, 512], f32, tag="odst")
                nc.scalar.activation(odt[:, 0:j1 - j0], pd[:, 0:j1 - j0],
                                     ACT.Identity, bias=t_db[:, 0:1])
                nc.sync.dma_start(outdenseT[:, j0:j1], odt[:, 0:j1 - j0])

            NDSEG = (NQ + 511) // 512

            # ---------------- geometry (per slab) ----------------
            def geometry(o0, Ws):  # generator: yields between segments
                """Compute w4b [128,3,Ws,4] bf16, zy/qoh [128,Ws,16] bf16
                for chunk columns [o0, o0+Ws)."""
                def gt(shape, dt_, tag):
                    return geo.tile(shape, dt_, name=f"{tag}_{o0}", tag=tag)

                dd = gt([128, 3, Ws], f32, "dd")
                sq3 = gt([128, 3, Ws], f32, "sq3")
                # pos tiles are [128, 4, W]; coord-major slab view
                pv = (W, 3), (1, Ws)
                TT(out=fl(dd, 3 * Ws), in0=sl(t_ps, o0, 0, *pv),
                   in1=sl(t_pq, o0, 0, *pv), op=ALU.subtract)
                TT(out=fl(sq3, 3 * Ws), in0=fl(dd, 3 * Ws), in1=fl(dd, 3 * Ws),
                   op=ALU.mult)

                def gW(tag):
                    return gt([128, Ws], f32, tag)

                xy2 = gW("xy2"); sq = gW("sq"); norm = gW("norm")
                nxy = gW("nxy"); azn = gW("azn"); den1 = gW("den1")
                rd1 = gW("rd1"); t1s = gW("t1s"); s1 = gW("s1")
                den2 = gW("den2"); rd2 = gW("rd2"); s2 = gW("s2")
                pole = gt([128, Ws], i32, "pole")
                wq = gW("wq"); zsg = gW("zsg"); zcp = gW("zcp")
                sqxy = gW("sqxy"); nrm = gW("nrm")
                abr = gt([128, Ws], i32, "abr")

                zofs = 2 * Ws
                yield
                TT(out=xy2[:], in0=sl(sq3, 0, Ws), in1=sl(sq3, Ws, Ws),
                   op=ALU.add)
                TT(out=sq[:], in0=xy2[:], in1=sl(sq3, zofs, Ws), op=ALU.add)
                AA(norm[:], sq[:], ACT.Sqrt)
                AA(nxy[:], xy2[:], ACT.Sqrt)
                AA(azn[:], sl(dd, zofs, Ws), ACT.Abs)
                STT(out=den1[:], in0=azn[:], scalar=EPS, in1=norm[:],
                    op0=ALU.add, op1=ALU.add)
                nc.vector.reciprocal_approx_fast(rd1[:], den1[:])
                TT(out=t1s[:], in0=norm[:], in1=rd1[:], op=ALU.mult)
                AA(s1[:], t1s[:], ACT.Sqrt, scale=3.0)
                TS(den2[:], nxy[:], EPS, None, op0=ALU.add)
                nc.vector.reciprocal_approx_fast(rd2[:], den2[:])
                TT(out=s2[:], in0=norm[:], in1=rd2[:], op=ALU.mult)
                yield
                STT(out=pole[:], in0=sl(sq3, zofs, Ws), scalar=1.25,
                    in1=xy2[:], op0=ALU.mult, op1=ALU.is_gt)
                nc.vector.tensor_copy(wq[:], s2[:])
                nc.vector.copy_predicated(wq[:], pole[:], s1[:])

                m3 = gt([128, 3, Ws], f32, "m3")
                AA(zsg[:], sl(dd, zofs, Ws), ACT.Sign)
                TT(out=zcp[:], in0=zsg[:], in1=norm[:], op=ALU.mult)
                TS(sl(m3, zofs, Ws), sl(dd, zofs, Ws), 1.5, None, op0=ALU.mult)
                nc.vector.copy_predicated(sl(m3, zofs, Ws), pole[:], zcp[:])

                yield
                xyc = gt([128, 2, Ws], f32, "xyc")
                TT(out=sl(xyc, 0, 0, (Ws, 2), (1, Ws)),
                   in0=sl(dd, 0, 0, (Ws, 2), (1, Ws)),
                   in1=sl(wq, 0, 0, (0, 2), (1, Ws)), op=ALU.mult)
                xyc2 = gt([128, 2, Ws], f32, "xyc2")
                TT(out=fl(xyc2, 2 * Ws), in0=fl(xyc, 2 * Ws),
                   in1=fl(xyc, 2 * Ws), op=ALU.mult)
                TT(out=sqxy[:], in0=sl(xyc2, 0, Ws), in1=sl(xyc2, Ws, Ws),
                   op=ALU.add)
                AA(nrm[:], sqxy[:], ACT.Sqrt)
                axy = gt([128, 2, Ws], f32, "axy")
                AA(fl(axy, 2 * Ws), fl(xyc, 2 * Ws), ACT.Abs)
                TT(out=abr[:], in0=sl(axy, Ws, Ws), in1=sl(axy, 0, Ws),
                   op=ALU.is_le)
                yield
                myx = gt([128, 2, Ws], f32, "myx")
                TS(fl(myx, 2 * Ws), fl(axy, 2 * Ws), EPS, None, op0=ALU.is_lt)
                sf = gt([128, 2, Ws], f32, "sf")
                TT(out=fl(sf, 2 * Ws), in0=fl(xyc, 2 * Ws), in1=fl(myx, 2 * Ws),
                   op=ALU.add)
                rsf = gt([128, 2, Ws], f32, "rsf")
                nc.vector.reciprocal_approx_fast(fl(rsf, 2 * Ws), fl(sf, 2 * Ws))
                rat = gt([128, 2, Ws], f32, "rat")
                TT(out=sl(rat, 0, Ws), in0=sl(xyc, 0, Ws), in1=sl(rsf, Ws, Ws),
                   op=ALU.mult)
                TT(out=sl(rat, Ws, Ws), in0=sl(xyc, Ws, Ws), in1=sl(rsf, 0, Ws),
                   op=ALU.mult)
                at12 = gt([128, 2, Ws], f32, "at12")
                AA(fl(at12, 2 * Ws), fl(rat, 2 * Ws), ACT.Arctan)
                sg = gt([128, 2, Ws], f32, "sg")
                AA(fl(sg, 2 * Ws), fl(xyc, 2 * Ws), ACT.Sign)
                yield
                tmpab = gt([128, 2, Ws], f32, "tmpab")
                TT(out=sl(tmpab, 0, 0, (Ws, 2), (1, Ws)),
                   in0=sl(sg, 0, 0, (Ws, 2), (1, Ws)),
                   in1=sl(nrm, 0, 0, (0, 2), (1, Ws)), op=ALU.mult)
                prod = gt([128, 2, Ws], f32, "prod")
                STT(out=sl(prod, 0, Ws), in0=sl(at12, 0, Ws), scalar=F4PI,
                    in1=sl(tmpab, Ws, Ws), op0=ALU.mult, op1=ALU.mult)
                STT(out=sl(prod, Ws, Ws), in0=sl(at12, Ws, Ws), scalar=F4PI,
                    in1=sl(tmpab, 0, Ws), op0=ALU.mult, op1=ALU.mult)
                nc.vector.tensor_copy(sl(m3, 0, Ws), sl(prod, 0, Ws))
                nc.vector.copy_predicated(sl(m3, 0, Ws), abr[:],
                                          sl(tmpab, 0, Ws))
                nc.vector.tensor_copy(sl(m3, Ws, Ws), sl(tmpab, Ws, Ws))
                nc.vector.copy_predicated(sl(m3, Ws, Ws), abr[:],
                                          sl(prod, Ws, Ws))

                # hat corner weights: w4[j] = relu(1 - |SC*m - (j-1.5)|)
                yield
                d4 = gt([128, 3 * Ws, 4], f32, "d4")
                nd4 = gt([128, 3 * Ws, 4], f32, "nd4")
                w4b = w4p.tile([128, 3, Ws, 4], bf16, name=f"w4b_{o0}", tag="w4b")
                zy = w4p.tile([128, Ws, 16], bf16, name=f"zy_{o0}", tag="zy")

                def hat(coord):
                    co = coord * Ws * 4
                    STT(out=sl(d4, co, 0, (4, Ws), (1, 4)),
                        in0=sl(m3, coord * Ws, 0, (1, Ws), (0, 4)),
                        scalar=SC,
                        in1=sl(io4m, 0, 0, (0, Ws), (1, 4)),
                        op0=ALU.mult, op1=ALU.subtract)
                    AA(sl(nd4, co, 4 * Ws), sl(d4, co, 4 * Ws), ACT.Abs)
                    AA(sl(w4b, co, 4 * Ws), sl(nd4, co, 4 * Ws), ACT.Relu,
                       bias=1.0, scale=-1.0)

                # z and y coords first so zy (the scatter payload) is ready
                # before the x-hat that only R needs
                hat(2)
                hat(1)
                TT(out=zy[:],
                   in0=sl(w4b, 2 * Ws * 4, 0, (4, Ws), (1, 4), (0, 4)),
                   in1=sl(w4b, 1 * Ws * 4, 0, (4, Ws), (0, 4), (1, 4)),
                   op=ALU.mult)
                hat(0)
                qoh = None
                if o0 + Ws == W:  # last slab: one-hot for DVE L fallback
                    qoh = w4p.tile([128, Ws, 16], bf16, name=f"qoh_{o0}",
                                   tag="qoh")
                    TT(out=qoh[:],
                       in0=sl(t_ql, o0, 0, (1, Ws), (0, 16)),
                       in1=sl(io16, 0, 0, (0, Ws), (1, 16)),
                       op=ALU.is_equal)
                slab_data[o0] = (w4b, zy, qoh)

            # ---------------- stage-1 + tap-GEMM (pipelined) -------------
            def tap_gemm(at_t, g):
                po = ps2.tile([COUT, 256], f32, space="PSUM", tag="tap")
                for t in range(16):
                    rhs = AP(at_t.tensor, at_t[:].offset + t,
                             [at_t[:].ap[0], [256, 16], [128, 2], [16, 8]])
                    nc.tensor.matmul(
                        out=po[:],
                        lhsT=t_g2r[:, t * 64:(t + 1) * 64],
                        rhs=rhs,
                        start=(t == 0), stop=(t == 15))
                ost = outp.tile([COUT, 256], f32, tag="ocst")
                nc.scalar.copy(ost[:], po[:])
                nc.sync.dma_start(outconvT[:, g * 256:(g + 1) * 256], ost[:])

            pend = None  # (at_t, g)
            slab_data = {}
            gens = [geometry(*sl_) for sl_ in SLABS]
            for _ in gens[0]:
                pass
            # feed slab k's geometry during the two groups before its slab,
            # spread over the preceding slab's groups
            NSEG = 7  # segments per geometry generator (6 yields + tail)
            feed = {}
            for si in range(1, len(SLABS)):
                start_g = SLABS[si][0] // 16
                prev_g0 = SLABS[si - 1][0] // 16
                span = max(1, start_g - prev_g0)
                for k in range(NSEG):
                    gg = prev_g0 + (k * span) // NSEG
                    feed.setdefault(gg, []).append(si)
            for g in range(NGRP):
                for si_ in feed.get(g, []):
                    next(gens[si_], None)
                si = SLAB_OF[g]
                o0, Ws = SLABS[si]
                if o0 not in slab_data:
                    for _ in gens[si]:
                        pass
                w4b, zy, qoh = slab_data[o0]
                lc = g * 16 - o0  # chunk offset within slab
                # R for 16 chunks in one DVE op: [128, 16, 128] bf16
                R_t = rpool.tile([128, 16, 128], bf16, tag="R")
                TT(out=sl(R_t, 0, 0, (128, 16), (32, 4), (1, 32)),
                   in0=sl(w4b, lc * 4, 0, (4, 16), (1, 4), (0, 32)),
                   in1=sl(t_f, (g * 16) * CIN, 0, (32, 16), (0, 4), (1, 32)),
                   op=ALU.mult)
                # L via gpsimd local_scatter (4 chunks/op); in the back
                # half DVE takes 1-2 quarters (it has slack there, and Pool
                # finishing early hides its slow drain under the tail)
                L_t = lpool.tile([128, 16, 256], bf16, tag="L")
                ndve = 0 if qoh is None else (2 if g == NGRP - 1 else 1)
                for j in range(4 - ndve):
                    c0 = g * 16 + 4 * j
                    nc.gpsimd.local_scatter(
                        out_ap=sl(L_t, j * 1024, 1024),
                        data_ap=sl(zy, (c0 - o0) * 16, 64),
                        idxs_ap=sl(t_si, c0 * 16, 64),
                        channels=128, num_elems=1024, num_idxs=64)
                if ndve:
                    j0_ = 4 - ndve
                    lq = lc + 4 * j0_
                    TT(out=sl(L_t, j0_ * 1024, 0,
                              (256, 4 * ndve), (16, 16), (1, 16)),
                       in0=sl(qoh, lq * 16, 0, (16, 4 * ndve), (1, 16), (0, 16)),
                       in1=sl(zy, lq * 16, 0, (16, 4 * ndve), (0, 16), (1, 16)),
                       op=ALU.mult)
                at_t = atp.tile([128, 16 * 256], f32r, tag="at")
                for j in range(4):
                    ps_t = ps1.tile([128, 1024], f32, space="PSUM", tag="s1")
                    for r in range(4):
                        k = j * 4 + r
                        nc.tensor.matmul(
                            out=ps_t[:, r * 256:(r + 1) * 256],
                            lhsT=R_t[:, k, :], rhs=L_t[:, k, :],
                            start=True, stop=True)
                    dst = at_t[:, j * 1024:(j + 1) * 1024]
                    if j == 0:
                        # DVE takes the FIRST quarter (ready earliest) so its
                        # in-order queue isn't blocked into the next group
                        nc.vector.tensor_copy(dst, ps_t[:])
                    else:
                        nc.scalar.copy(dst, ps_t[:])
                if pend is not None:
                    tap_gemm(*pend)
                pend = (at_t, g)
                if 1 <= g <= NDSEG:
                    dense_seg(g - 1)
            tap_gemm(*pend)
            for s_ in range(min(NGRP - 1, NDSEG), NDSEG):
                dense_seg(s_)

    nc.compile()
    return nc


# ----------------------------------------------------------------------------
# Entry point
# ----------------------------------------------------------------------------
def _prepare(feats, pos, filt, dense_w, dense_b, src_idx, qry_idx):
    feats = np.ascontiguousarray(np.asarray(feats, np.float32))
    pos = np.ascontiguousarray(np.asarray(pos, np.float32))
    filt = np.asarray(filt, np.float32)
    dense_w = np.asarray(dense_w, np.float32)
    dense_b = np.asarray(dense_b, np.float32)
    src_idx = np.asarray(src_idx).astype(np.int64)
    qry_idx = np.asarray(qry_idx).astype(np.int64)

    plans, bstart, bsz = _plan(qry_idx)
    NCH = max(len(p['chunks']) for p in plans)
    NCHP = ((NCH + 15) // 16) * 16
    NQ = NCHP * 16

    feats_bf = feats.astype(BF16)

    # filter regroup: G2[ax*32+c, t*64+o] = filt[az, ay, ax, c, o], t = az*4+ay
    G2 = np.zeros((128, 16 * 64), np.float32)
    for az in range(4):
        for ay in range(4):
            t = az * 4 + ay
            for ax in range(4):
                G2[ax * 32:(ax + 1) * 32, t * 64:(t + 1) * 64] = filt[az, ay, ax]

    in_maps = []
    for c, p in enumerate(plans):
        possrc, posqry, fsrc, sidx, qlocf = _pack_core(
            p, bstart, pos, feats_bf, qry_idx, src_idx, NCHP)
        ftT = np.zeros((CIN, NQ), BF16)
        ftT[:, 0:p['nq']] = feats_bf[p['q0']:p['q0'] + p['nq']].T
        in_maps.append({
            "possrc": possrc, "posqry": posqry, "fsrc": fsrc, "sidx": sidx,
            "qlocf": qlocf,
            "g2": G2, "featsT": ftT, "denw": dense_w.astype(BF16),
            "denb": dense_b.reshape(COUT, 1).astype(np.float32),
        })

    key = (NCHP, NQ)
    if key not in _COMPILED:
        _COMPILED[key] = _build_bass(NCHP, NQ)
    nc = _COMPILED[key]
    return nc, in_maps, plans


def kernel(feats, pos, filt, dense_w, dense_b, src_idx, qry_idx):
    from concourse.bass_utils import run_bass_kernel_spmd

    nc, in_maps, plans = _prepare(feats, pos, filt, dense_w, dense_b,
                                  src_idx, qry_idx)
    res = run_bass_kernel_spmd(nc, in_maps, core_ids=list(range(NCORES)))

    ans_conv = np.zeros((N, COUT), np.float32)
    ans_dense = np.zeros((N, COUT), np.float32)
    for c, p in enumerate(plans):
        outT = res.results[c]["outconvT"]
        for ci, (bA, bB) in enumerate(p['chunks']):
            for half, b in enumerate((bA, bB)):
                if b is None:
                    continue
                cols = ci * 16 + half * 8
                ans_conv[8 * b:8 * b + 8] = outT[:, cols:cols + 8].T
        dT = res.results[c]["outdenseT"]
        ans_dense[p['q0']:p['q0'] + p['nq']] = dT[:, 0:p['nq']].T
    return ans_conv, ans_dense


# revision 40
# speedup vs baseline: 1.0015x; 1.0015x over previous
"""Trainium2 Bass kernel for nn_ContinuousConvolutionBlock (gnn_message_passing).

Strategy (per sharding hint: partition points across 8 cores; each core owns its
queries' scatter-reduce and tap-GEMM; filter + dense weights replicated):

Host side (index plumbing / input marshalling only — zero FLOPs):
  - qry_idx is sorted; queries are grouped into 8-query blocks, blocks paired
    into 128-edge-slot "chunks" (two-pointer bin packing, ~3% padding).
  - Consecutive block ranges are assigned to the 8 cores; per-core per-slot
    payload arrays (pos[src]/pos[qry] coord-major, feats[src] bf16, int16
    scatter indices qloc*16+t, local query ids) are marshalled and DMA'd.

Device side (all FLOP-bearing compute), spread over all five engines:
  - Geometry (DVE+ACT): ball->cube volume-preserving map on UNSCALED deltas
    (map is linear in scale; 2/EXTENT folds into the final grid transform),
    computed in two W-slabs so the group pipeline starts after slab 0;
    4-wide trilinear corner weights via the hat identity
    w4[j] = relu(1 - |g - j|).
  - L build: L[slot, (q16, az, ay)] = one-hot scatter of zy = w4z (x) w4y
    into the query slot — 3 of 4 quarters via gpsimd local_scatter with
    host-precomputed indices (zeroes dst, skips -1 padding), 1 quarter via
    DVE broadcast-mult with a query one-hot (engine balance).
  - R build (DVE): R[slot, (ax,c)] = w4x (x) feats, bf16, 16 chunks/op.
  - Stage-1 (PE): A^T[(ax,c), (q,az,ay)] = R^T @ L per chunk, bf16,
    fp32 PSUM accumulate; PSUM staged to SBUF as bf16 on ACT (3) + DVE (1).
  - Tap-GEMM (PE): out^T += G_t^T @ A^T-slices over 16 taps, bf16, fused
    over group PAIRS (512-wide moving dim) and software-pipelined one pair
    behind stage-1 so the PE stream stays dense.
  - Dense branch (PE, bf16) issued first so PE warms during the prologue.
  Outputs are produced transposed ([64, nq]); host transposes/reorders back.
"""
import sys
import os
sys.path.insert(0, '/opt/trn_rl_repo')
import numpy as np
import ml_dtypes

BF16 = ml_dtypes.bfloat16

N = 30000
CIN = 32
COUT = 64
KS = 4
EXTENT = 0.08
NCORES = 8
NBLK = N // 8  # 3750 eight-query blocks

_COMPILED = {}


# ----------------------------------------------------------------------------
# Host planning
# ----------------------------------------------------------------------------
def _plan(qry_idx):
    deg = np.bincount(qry_idx, minlength=N)
    bsz = deg.reshape(NBLK, 8).sum(1)
    bstart = np.concatenate([[0], np.cumsum(bsz)]).astype(np.int64)
    per = [NBLK // NCORES + (1 if c < NBLK % NCORES else 0) for c in range(NCORES)]
    b0 = np.concatenate([[0], np.cumsum(per)]).astype(np.int64)
    plans = []
    for c in range(NCORES):
        blocks = list(range(b0[c], b0[c + 1]))
        asc = sorted(blocks, key=lambda b: bsz[b])
        chunks = []
        lo, hi = 0, len(asc) - 1
        while lo <= hi:
            if lo == hi:
                chunks.append((asc[hi], None)); break
            if bsz[asc[hi]] + bsz[asc[lo]] <= 128:
                chunks.append((asc[hi], asc[lo])); hi -= 1; lo += 1
            else:
                chunks.append((asc[hi], None)); hi -= 1
        plans.append(dict(blocks=blocks, chunks=chunks, q0=int(8 * b0[c]),
                          nq=int(8 * (b0[c + 1] - b0[c]))))
    return plans, bstart, bsz


def _pack_core(plan_c, bstart, pos, feats_bf, qry_idx, src_idx, NCHP):
    """Per-slot payload: pos coord-major, feats bf16, int16 scatter indices."""
    possrc = np.zeros((128, 4, NCHP), np.float32)
    posqry = np.zeros((128, 4, NCHP), np.float32)
    fsrc = np.zeros((128, NCHP, CIN), BF16)
    qloc = np.full((128, NCHP), -1, np.int32)
    for ci, (bA, bB) in enumerate(plan_c['chunks']):
        s = 0
        for half, b in enumerate((bA, bB)):
            if b is None:
                continue
            e0, e1 = int(bstart[b]), int(bstart[b + 1])
            n = e1 - e0
            sl = slice(s, s + n)
            possrc[sl, 0:3, ci] = pos[src_idx[e0:e1]]
            posqry[sl, 0:3, ci] = pos[qry_idx[e0:e1]]
            fsrc[sl, ci, :] = feats_bf[src_idx[e0:e1]]
            qloc[sl, ci] = (qry_idx[e0:e1] - 8 * b) + 8 * half
            s += n
    # scatter index: within each 4-chunk scatter window,
    # idx[slot, ci, t] = (ci%4)*256 + qloc*16 + t  (or -1 for padding)
    t16 = np.arange(16, dtype=np.int32)
    idx = ((np.arange(NCHP, dtype=np.int32)[None, :] % 4) * 256
           + qloc * 16)[:, :, None] + t16[None, None, :]
    idx = np.where((qloc < 0)[:, :, None], -1, idx).astype(np.int16)
    return possrc, posqry, fsrc, idx, qloc.astype(np.float32)


# ----------------------------------------------------------------------------
# Device kernel
# ----------------------------------------------------------------------------
def _build_bass(NCHP, NQ):
    import concourse.bass as bass
    import concourse.tile as tile
    from concourse import bacc, mybir
    from concourse.bass import AP

    f32 = mybir.dt.float32
    f32r = mybir.dt.float32r
    bf16 = mybir.dt.bfloat16
    i32 = mybir.dt.int32
    i16 = mybir.dt.int16
    ALU = mybir.AluOpType
    ACT = mybir.ActivationFunctionType
    EPS = 1e-12
    F4PI = float(4.0 / np.pi)
    SC = 1.5 * (2.0 / EXTENT)  # grid scale folded with coord normalization

    nc = bacc.Bacc("TRN2", target_bir_lowering=False, debug=False)

    W = NCHP
    NGRP = W // 16
    # uneven slab splits (at group boundaries): a tiny first slab so the
    # scatter/matmul pipeline starts early; later slabs' geometry is fed
    # incrementally into DVE slack during earlier groups
    gb = sorted(set(min(x, NGRP) for x in (0, 8, NGRP)))
    SLABS = [(gb[i] * 16, (gb[i + 1] - gb[i]) * 16)
             for i in range(len(gb) - 1) if gb[i + 1] > gb[i]]
    SLAB_OF = {}
    for si, (o0, Ws) in enumerate(SLABS):
        for g in range(o0 // 16, (o0 + Ws) // 16):
            SLAB_OF[g] = si

    possrc = nc.dram_tensor("possrc", (128, 4, W), f32, kind="ExternalInput")
    posqry = nc.dram_tensor("posqry", (128, 4, W), f32, kind="ExternalInput")
    fsrc = nc.dram_tensor("fsrc", (128, W, CIN), bf16, kind="ExternalInput")
    sidx = nc.dram_tensor("sidx", (128, W, 16), i16, kind="ExternalInput")
    qlocf = nc.dram_tensor("qlocf", (128, W), f32, kind="ExternalInput")
    g2 = nc.dram_tensor("g2", (128, 16 * 64), f32, kind="ExternalInput")
    featsT = nc.dram_tensor("featsT", (CIN, NQ), bf16, kind="ExternalInput")
    denw = nc.dram_tensor("denw", (CIN, COUT), bf16, kind="ExternalInput")
    denb = nc.dram_tensor("denb", (COUT, 1), f32, kind="ExternalInput")

    outconvT = nc.dram_tensor("outconvT", (COUT, NQ), f32, kind="ExternalOutput")
    outdenseT = nc.dram_tensor("outdenseT", (COUT, NQ), f32, kind="ExternalOutput")

    with tile.TileContext(nc) as tc:
        with tc.tile_pool(name="inp", bufs=1) as inp, \
             tc.tile_pool(name="geo", bufs=1) as geo, \
             tc.tile_pool(name="w4p", bufs=2) as w4p, \
             tc.tile_pool(name="tmp", bufs=1) as tmp, \
             tc.tile_pool(name="lp", bufs=3) as lpool, \
             tc.tile_pool(name="rp", bufs=3) as rpool, \
             tc.tile_pool(name="at", bufs=2) as atp, \
             tc.tile_pool(name="outp", bufs=2) as outp, \
             tc.tile_pool(name="ps1", bufs=3, space="PSUM") as ps1, \
             tc.tile_pool(name="ps2", bufs=1, space="PSUM") as ps2, \
             tc.tile_pool(name="ps3", bufs=1, space="PSUM") as ps3:

            # ---------------- input DMAs ----------------
            t_ps = inp.tile([128, 4, W], f32)
            t_pq = inp.tile([128, 4, W], f32)
            t_f = inp.tile([128, W, CIN], bf16)
            t_si = inp.tile([128, W, 16], i16)
            t_ql = inp.tile([128, W], f32)
            t_g2 = inp.tile([128, 16 * 64], f32)
            t_ftT = inp.tile([CIN, NQ], bf16)
            t_dw = inp.tile([CIN, COUT], bf16)
            t_db = inp.tile([COUT, 1], f32)
            # slab-0 pos first so geometry can start early
            W0 = SLABS[0][1]
            nc.sync.dma_start(t_ps[:, :, 0:W0], possrc[:, :, 0:W0])
            nc.sync.dma_start(t_pq[:, :, 0:W0], posqry[:, :, 0:W0])
            nc.sync.dma_start(t_si[:], sidx[:])
            nc.sync.dma_start(t_f[:], fsrc[:])
            nc.sync.dma_start(t_dw[:], denw[:])
            nc.sync.dma_start(t_db[:], denb[:])
            nc.sync.dma_start(t_ps[:, :, W0:W], possrc[:, :, W0:W])
            nc.sync.dma_start(t_pq[:, :, W0:W], posqry[:, :, W0:W])
            nc.sync.dma_start(t_ftT[:], featsT[:])
            nc.sync.dma_start(t_g2[:], g2[:])
            nc.sync.dma_start(t_ql[:], qlocf[:])

            # iota constants: io4m = j - 1.5 (j=0..3), io16 = 0..15
            io4i = tmp.tile([128, 4], i32)
            nc.gpsimd.iota(io4i[:], pattern=[[1, 4]], base=0, channel_multiplier=0)
            io4m = tmp.tile([128, 4], f32)
            nc.scalar.activation(io4m[:], io4i[:], ACT.Copy, bias=-1.5)
            t_g2r = inp.tile([128, 16 * 64], f32r)
            nc.vector.tensor_copy(t_g2r[:], t_g2[:])
            io16i = tmp.tile([128, 16], i32)
            nc.gpsimd.iota(io16i[:], pattern=[[1, 16]], base=0,
                           channel_multiplier=0)
            io16 = tmp.tile([128, 16], f32)
            nc.scalar.activation(io16[:], io16i[:], ACT.Copy)

            TT = nc.vector.tensor_tensor
            TS = nc.vector.tensor_scalar
            STT = nc.vector.scalar_tensor_tensor
            AA = nc.scalar.activation

            def fl(t, n, off=0):  # flat [128, n] view
                return AP(t.tensor, t[:].offset + off, [t[:].ap[0], [1, n]])

            def sl(t, off, n, *dims):  # strided view: dims = (stride, count)*
                pat = [t[:].ap[0]] + [[s, c] for (s, c) in dims] if dims else \
                      [t[:].ap[0], [1, n]]
                return AP(t.tensor, t[:].offset + off, pat)

            # ------------- dense branch, issued one segment per group ------
            def dense_seg(s):
                j0 = s * 512
                j1 = min(NQ, j0 + 512)
                pd = ps3.tile([COUT, 512], f32, space="PSUM", tag="den")
                nc.tensor.matmul(
                    out=pd[:, 0:j1 - j0],
                    lhsT=t_dw[:],
                    rhs=t_ftT[:, j0:j1],
                    start=True, stop=True)
                odt = outp.tile([COUT, 512], f32, tag="odst")
                nc.scalar.activation(odt[:, 0:j1 - j0], pd[:, 0:j1 - j0],
                                     ACT.Identity, bias=t_db[:, 0:1])
                nc.sync.dma_start(outdenseT[:, j0:j1], odt[:, 0:j1 - j0])

            NDSEG = (NQ + 511) // 512

            # ---------------- geometry (per slab) ----------------
            def geometry(o0, Ws):  # generator: yields between segments
                """Compute w4b [128,3,Ws,4] bf16, zy/qoh [128,Ws,16] bf16
                for chunk columns [o0, o0+Ws)."""
                def gt(shape, dt_, tag):
                    return geo.tile(shape, dt_, name=f"{tag}_{o0}", tag=tag)

                dd = gt([128, 3, Ws], f32, "dd")
                sq3 = gt([128, 3, Ws], f32, "sq3")
                # pos tiles are [128, 4, W]; coord-major slab view
                pv = (W, 3), (1, Ws)
                TT(out=fl(dd, 3 * Ws), in0=sl(t_ps, o0, 0, *pv),
                   in1=sl(t_pq, o0, 0, *pv), op=ALU.subtract)
                TT(out=fl(sq3, 3 * Ws), in0=fl(dd, 3 * Ws), in1=fl(dd, 3 * Ws),
                   op=ALU.mult)

                def gW(tag):
                    return gt([128, Ws], f32, tag)

                xy2 = gW("xy2"); sq = gW("sq"); norm = gW("norm")
                nxy = gW("nxy"); azn = gW("azn"); den1 = gW("den1")
                rd1 = gW("rd1"); t1s = gW("t1s"); s1 = gW("s1")
                den2 = gW("den2"); rd2 = gW("rd2"); s2 = gW("s2")
                pole = gt([128, Ws], i32, "pole")
                wq = gW("wq"); zsg = gW("zsg"); zcp = gW("zcp")
                sqxy = gW("sqxy"); nrm = gW("nrm")
                abr = gt([128, Ws], i32, "abr")

                zofs = 2 * Ws
                yield
                TT(out=xy2[:], in0=sl(sq3, 0, Ws), in1=sl(sq3, Ws, Ws),
                   op=ALU.add)
                TT(out=sq[:], in0=xy2[:], in1=sl(sq3, zofs, Ws), op=ALU.add)
                AA(norm[:], sq[:], ACT.Sqrt)
                AA(nxy[:], xy2[:], ACT.Sqrt)
                AA(azn[:], sl(dd, zofs, Ws), ACT.Abs)
                STT(out=den1[:], in0=azn[:], scalar=EPS, in1=norm[:],
                    op0=ALU.add, op1=ALU.add)
                nc.vector.reciprocal_approx_fast(rd1[:], den1[:])
                TT(out=t1s[:], in0=norm[:], in1=rd1[:], op=ALU.mult)
                AA(s1[:], t1s[:], ACT.Sqrt, scale=3.0)
                TS(den2[:], nxy[:], EPS, None, op0=ALU.add)
                nc.vector.reciprocal_approx_fast(rd2[:], den2[:])
                TT(out=s2[:], in0=norm[:], in1=rd2[:], op=ALU.mult)
                yield
                STT(out=pole[:], in0=sl(sq3, zofs, Ws), scalar=1.25,
                    in1=xy2[:], op0=ALU.mult, op1=ALU.is_gt)
                nc.vector.tensor_copy(wq[:], s2[:])
                nc.vector.copy_predicated(wq[:], pole[:], s1[:])

                m3 = gt([128, 3, Ws], f32, "m3")
                AA(zsg[:], sl(dd, zofs, Ws), ACT.Sign)
                TT(out=zcp[:], in0=zsg[:], in1=norm[:], op=ALU.mult)
                TS(sl(m3, zofs, Ws), sl(dd, zofs, Ws), 1.5, None, op0=ALU.mult)
                nc.vector.copy_predicated(sl(m3, zofs, Ws), pole[:], zcp[:])

                yield
                xyc = gt([128, 2, Ws], f32, "xyc")
                TT(out=sl(xyc, 0, 0, (Ws, 2), (1, Ws)),
                   in0=sl(dd, 0, 0, (Ws, 2), (1, Ws)),
                   in1=sl(wq, 0, 0, (0, 2), (1, Ws)), op=ALU.mult)
                xyc2 = gt([128, 2, Ws], f32, "xyc2")
                TT(out=fl(xyc2, 2 * Ws), in0=fl(xyc, 2 * Ws),
                   in1=fl(xyc, 2 * Ws), op=ALU.mult)
                TT(out=sqxy[:], in0=sl(xyc2, 0, Ws), in1=sl(xyc2, Ws, Ws),
                   op=ALU.add)
                AA(nrm[:], sqxy[:], ACT.Sqrt)
                axy = gt([128, 2, Ws], f32, "axy")
                AA(fl(axy, 2 * Ws), fl(xyc, 2 * Ws), ACT.Abs)
                TT(out=abr[:], in0=sl(axy, Ws, Ws), in1=sl(axy, 0, Ws),
                   op=ALU.is_le)
                yield
                myx = gt([128, 2, Ws], f32, "myx")
                TS(fl(myx, 2 * Ws), fl(axy, 2 * Ws), EPS, None, op0=ALU.is_lt)
                sf = gt([128, 2, Ws], f32, "sf")
                TT(out=fl(sf, 2 * Ws), in0=fl(xyc, 2 * Ws), in1=fl(myx, 2 * Ws),
                   op=ALU.add)
                rsf = gt([128, 2, Ws], f32, "rsf")
                nc.vector.reciprocal_approx_fast(fl(rsf, 2 * Ws), fl(sf, 2 * Ws))
                rat = gt([128, 2, Ws], f32, "rat")
                TT(out=sl(rat, 0, Ws), in0=sl(xyc, 0, Ws), in1=sl(rsf, Ws, Ws),
                   op=ALU.mult)
                TT(out=sl(rat, Ws, Ws), in0=sl(xyc, Ws, Ws), in1=sl(rsf, 0, Ws),
                   op=ALU.mult)
                at12 = gt([128, 2, Ws], f32, "at12")
                AA(fl(at12, 2 * Ws), fl(rat, 2 * Ws), ACT.Arctan)
                sg = gt([128, 2, Ws], f32, "sg")
                AA(fl(sg, 2 * Ws), fl(xyc, 2 * Ws), ACT.Sign)
                yield
                tmpab = gt([128, 2, Ws], f32, "tmpab")
                TT(out=sl(tmpab, 0, 0, (Ws, 2), (1, Ws)),
                   in0=sl(sg, 0, 0, (Ws, 2), (1, Ws)),
                   in1=sl(nrm, 0, 0, (0, 2), (1, Ws)), op=ALU.mult)
                prod = gt([128, 2, Ws], f32, "prod")
                STT(out=sl(prod, 0, Ws), in0=sl(at12, 0, Ws), scalar=F4PI,
                    in1=sl(tmpab, Ws, Ws), op0=ALU.mult, op1=ALU.mult)
                STT(out=sl(prod, Ws, Ws), in0=sl(at12, Ws, Ws), scalar=F4PI,
                    in1=sl(tmpab, 0, Ws), op0=ALU.mult, op1=ALU.mult)
                nc.vector.tensor_copy(sl(m3, 0, Ws), sl(prod, 0, Ws))
                nc.vector.copy_predicated(sl(m3, 0, Ws), abr[:],
                                          sl(tmpab, 0, Ws))
                nc.vector.tensor_copy(sl(m3, Ws, Ws), sl(tmpab, Ws, Ws))
                nc.vector.copy_predicated(sl(m3, Ws, Ws), abr[:],
                                          sl(prod, Ws, Ws))

                # hat corner weights: w4[j] = relu(1 - |SC*m - (j-1.5)|)
                yield
                d4 = gt([128, 3 * Ws, 4], f32, "d4")
                nd4 = gt([128, 3 * Ws, 4], f32, "nd4")
                w4b = w4p.tile([128, 3, Ws, 4], bf16, name=f"w4b_{o0}", tag="w4b")
                zy = w4p.tile([128, Ws, 16], bf16, name=f"zy_{o0}", tag="zy")

                def hat(coord):
                    co = coord * Ws * 4
                    STT(out=sl(d4, co, 0, (4, Ws), (1, 4)),
                        in0=sl(m3, coord * Ws, 0, (1, Ws), (0, 4)),
                        scalar=SC,
                        in1=sl(io4m, 0, 0, (0, Ws), (1, 4)),
                        op0=ALU.mult, op1=ALU.subtract)
                    AA(sl(nd4, co, 4 * Ws), sl(d4, co, 4 * Ws), ACT.Abs)
                    AA(sl(w4b, co, 4 * Ws), sl(nd4, co, 4 * Ws), ACT.Relu,
                       bias=1.0, scale=-1.0)

                # z and y coords first so zy (the scatter payload) is ready
                # before the x-hat that only R needs
                hat(2)
                hat(1)
                TT(out=zy[:],
                   in0=sl(w4b, 2 * Ws * 4, 0, (4, Ws), (1, 4), (0, 4)),
                   in1=sl(w4b, 1 * Ws * 4, 0, (4, Ws), (0, 4), (1, 4)),
                   op=ALU.mult)
                hat(0)
                qoh = None
                if o0 + Ws == W:  # last slab: one-hot for DVE L fallback
                    qoh = w4p.tile([128, Ws, 16], bf16, name=f"qoh_{o0}",
                                   tag="qoh")
                    TT(out=qoh[:],
                       in0=sl(t_ql, o0, 0, (1, Ws), (0, 16)),
                       in1=sl(io16, 0, 0, (0, Ws), (1, 16)),
                       op=ALU.is_equal)
                slab_data[o0] = [w4b, zy, qoh]

            # ---------------- stage-1 + tap-GEMM (pipelined) -------------
            def tap_gemm(at_t, g):
                po = ps2.tile([COUT, 256], f32, space="PSUM", tag="tap")
                for t in range(16):
                    rhs = AP(at_t.tensor, at_t[:].offset + t,
                             [at_t[:].ap[0], [256, 16], [128, 2], [16, 8]])
                    nc.tensor.matmul(
                        out=po[:],
                        lhsT=t_g2r[:, t * 64:(t + 1) * 64],
                        rhs=rhs,
                        start=(t == 0), stop=(t == 15))
                ost = outp.tile([COUT, 256], f32, tag="ocst")
                nc.scalar.copy(ost[:], po[:])
                nc.sync.dma_start(outconvT[:, g * 256:(g + 1) * 256], ost[:])

            pend = None  # (at_t, g)
            slab_data = {}
            gens = [geometry(*sl_) for sl_ in SLABS]
            for _ in gens[0]:
                pass
            # feed slab k's geometry during the two groups before its slab,
            # spread over the preceding slab's groups
            NSEG = 7  # segments per geometry generator (6 yields + tail)
            feed = {}
            for si in range(1, len(SLABS)):
                start_g = SLABS[si][0] // 16
                prev_g0 = SLABS[si - 1][0] // 16
                span = max(1, start_g - prev_g0)
                for k in range(NSEG):
                    gg = prev_g0 + (k * span) // NSEG
                    feed.setdefault(gg, []).append(si)
            for g in range(NGRP):
                for si_ in feed.get(g, []):
                    next(gens[si_], None)
                si = SLAB_OF[g]
                o0, Ws = SLABS[si]
                if o0 not in slab_data:
                    for _ in gens[si]:
                        pass
                if g == 1 and len(SLABS) > 1:
                    # lazily build slab-0's query one-hot inside group 1's
                    # Pool-bound window so groups 2+ can shed a scatter to DVE
                    sd0 = slab_data[SLABS[0][0]]
                    if sd0[2] is None:
                        o00, Ws0 = SLABS[0]
                        q0t = w4p.tile([128, Ws0, 16], bf16, name="qoh0",
                                       tag="qoh0")
                        TT(out=q0t[:],
                           in0=sl(t_ql, o00, 0, (1, Ws0), (0, 16)),
                           in1=sl(io16, 0, 0, (0, Ws0), (1, 16)),
                           op=ALU.is_equal)
                        sd0[2] = q0t
                w4b, zy, qoh = slab_data[o0]
                lc = g * 16 - o0  # chunk offset within slab
                # R for 16 chunks in one DVE op: [128, 16, 128] bf16
                R_t = rpool.tile([128, 16, 128], bf16, tag="R")
                TT(out=sl(R_t, 0, 0, (128, 16), (32, 4), (1, 32)),
                   in0=sl(w4b, lc * 4, 0, (4, 16), (1, 4), (0, 32)),
                   in1=sl(t_f, (g * 16) * CIN, 0, (32, 16), (0, 4), (1, 32)),
                   op=ALU.mult)
                # L via gpsimd local_scatter (4 chunks/op); in the back
                # half DVE takes 1-2 quarters (it has slack there, and Pool
                # finishing early hides its slow drain under the tail)
                L_t = lpool.tile([128, 16, 256], bf16, tag="L")
                ndve = 0 if (qoh is None or g < 2) else \
                    (2 if g == NGRP - 1 else 1)
                act_all = 2 <= g < 8  # ACT takes all 4 PSUM copies here
                for j in range(4 - ndve):
                    c0 = g * 16 + 4 * j
                    nc.gpsimd.local_scatter(
                        out_ap=sl(L_t, j * 1024, 1024),
                        data_ap=sl(zy, (c0 - o0) * 16, 64),
                        idxs_ap=sl(t_si, c0 * 16, 64),
                        channels=128, num_elems=1024, num_idxs=64)
                if ndve:
                    j0_ = 4 - ndve
                    lq = lc + 4 * j0_
                    TT(out=sl(L_t, j0_ * 1024, 0,
                              (256, 4 * ndve), (16, 16), (1, 16)),
                       in0=sl(qoh, lq * 16, 0, (16, 4 * ndve), (1, 16), (0, 16)),
                       in1=sl(zy, lq * 16, 0, (16, 4 * ndve), (0, 16), (1, 16)),
                       op=ALU.mult)
                at_t = atp.tile([128, 16 * 256], f32r, tag="at")
                for j in range(4):
                    ps_t = ps1.tile([128, 1024], f32, space="PSUM", tag="s1")
                    for r in range(4):
                        k = j * 4 + r
                        nc.tensor.matmul(
                            out=ps_t[:, r * 256:(r + 1) * 256],
                            lhsT=R_t[:, k, :], rhs=L_t[:, k, :],
                            start=True, stop=True)
                    dst = at_t[:, j * 1024:(j + 1) * 1024]
                    if j == 0 and not act_all:
                        # DVE takes the FIRST quarter (ready earliest) so its
                        # in-order queue isn't blocked into the next group
                        nc.vector.tensor_copy(dst, ps_t[:])
                    else:
                        nc.scalar.copy(dst, ps_t[:])
                if pend is not None:
                    tap_gemm(*pend)
                pend = (at_t, g)
                if 1 <= g <= NDSEG:
                    dense_seg(g - 1)
            tap_gemm(*pend)
            for s_ in range(min(NGRP - 1, NDSEG), NDSEG):
                dense_seg(s_)

    nc.compile()
    return nc


# ----------------------------------------------------------------------------
# Entry point
# ----------------------------------------------------------------------------
def _prepare(feats, pos, filt, dense_w, dense_b, src_idx, qry_idx):
    feats = np.ascontiguousarray(np.asarray(feats, np.float32))
    pos = np.ascontiguousarray(np.asarray(pos, np.float32))
    filt = np.asarray(filt, np.float32)
    dense_w = np.asarray(dense_w, np.float32)
    dense_b = np.asarray(dense_b, np.float32)
    src_idx = np.asarray(src_idx).astype(np.int64)
    qry_idx = np.asarray(qry_idx).astype(np.int64)

    plans, bstart, bsz = _plan(qry_idx)
    NCH = max(len(p['chunks']) for p in plans)
    NCHP = ((NCH + 15) // 16) * 16
    NQ = NCHP * 16

    feats_bf = feats.astype(BF16)

    # filter regroup: G2[ax*32+c, t*64+o] = filt[az, ay, ax, c, o], t = az*4+ay
    G2 = np.zeros((128, 16 * 64), np.float32)
    for az in range(4):
        for ay in range(4):
            t = az * 4 + ay
            for ax in range(4):
                G2[ax * 32:(ax + 1) * 32, t * 64:(t + 1) * 64] = filt[az, ay, ax]

    in_maps = []
    for c, p in enumerate(plans):
        possrc, posqry, fsrc, sidx, qlocf = _pack_core(
            p, bstart, pos, feats_bf, qry_idx, src_idx, NCHP)
        ftT = np.zeros((CIN, NQ), BF16)
        ftT[:, 0:p['nq']] = feats_bf[p['q0']:p['q0'] + p['nq']].T
        in_maps.append({
            "possrc": possrc, "posqry": posqry, "fsrc": fsrc, "sidx": sidx,
            "qlocf": qlocf,
            "g2": G2, "featsT": ftT, "denw": dense_w.astype(BF16),
            "denb": dense_b.reshape(COUT, 1).astype(np.float32),
        })

    key = (NCHP, NQ)
    if key not in _COMPILED:
        _COMPILED[key] = _build_bass(NCHP, NQ)
    nc = _COMPILED[key]
    return nc, in_maps, plans


def kernel(feats, pos, filt, dense_w, dense_b, src_idx, qry_idx):
    from concourse.bass_utils import run_bass_kernel_spmd

    nc, in_maps, plans = _prepare(feats, pos, filt, dense_w, dense_b,
                                  src_idx, qry_idx)
    res = run_bass_kernel_spmd(nc, in_maps, core_ids=list(range(NCORES)))

    ans_conv = np.zeros((N, COUT), np.float32)
    ans_dense = np.zeros((N, COUT), np.float32)
    for c, p in enumerate(plans):
        outT = res.results[c]["outconvT"]
        for ci, (bA, bB) in enumerate(p['chunks']):
            for half, b in enumerate((bA, bB)):
                if b is None:
                    continue
                cols = ci * 16 + half * 8
                ans_conv[8 * b:8 * b + 8] = outT[:, cols:cols + 8].T
        dT = res.results[c]["outdenseT"]
        ans_dense[p['q0']:p['q0'] + p['nq']] = dT[:, 0:p['nq']].T
    return ans_conv, ans_dense


# revision 42
# speedup vs baseline: 1.0087x; 1.0072x over previous
"""Trainium2 Bass kernel for nn_ContinuousConvolutionBlock (gnn_message_passing).

Strategy (per sharding hint: partition points across 8 cores; each core owns its
queries' scatter-reduce and tap-GEMM; filter + dense weights replicated):

Host side (index plumbing / input marshalling only — zero FLOPs):
  - qry_idx is sorted; queries are grouped into 8-query blocks, blocks paired
    into 128-edge-slot "chunks" (two-pointer bin packing, ~3% padding).
  - Consecutive block ranges are assigned to the 8 cores; per-core per-slot
    payload arrays (pos[src]/pos[qry] coord-major, feats[src] bf16, int16
    scatter indices qloc*16+t, local query ids) are marshalled and DMA'd.

Device side (all FLOP-bearing compute), spread over all five engines:
  - Geometry (DVE+ACT): ball->cube volume-preserving map on UNSCALED deltas
    (map is linear in scale; 2/EXTENT folds into the final grid transform),
    computed in two W-slabs so the group pipeline starts after slab 0;
    4-wide trilinear corner weights via the hat identity
    w4[j] = relu(1 - |g - j|).
  - L build: L[slot, (q16, az, ay)] = one-hot scatter of zy = w4z (x) w4y
    into the query slot — 3 of 4 quarters via gpsimd local_scatter with
    host-precomputed indices (zeroes dst, skips -1 padding), 1 quarter via
    DVE broadcast-mult with a query one-hot (engine balance).
  - R build (DVE): R[slot, (ax,c)] = w4x (x) feats, bf16, 16 chunks/op.
  - Stage-1 (PE): A^T[(ax,c), (q,az,ay)] = R^T @ L per chunk, bf16,
    fp32 PSUM accumulate; PSUM staged to SBUF as bf16 on ACT (3) + DVE (1).
  - Tap-GEMM (PE): out^T += G_t^T @ A^T-slices over 16 taps, bf16, fused
    over group PAIRS (512-wide moving dim) and software-pipelined one pair
    behind stage-1 so the PE stream stays dense.
  - Dense branch (PE, bf16) issued first so PE warms during the prologue.
  Outputs are produced transposed ([64, nq]); host transposes/reorders back.
"""
import sys
import os
sys.path.insert(0, '/opt/trn_rl_repo')
import numpy as np
import ml_dtypes

BF16 = ml_dtypes.bfloat16

N = 30000
CIN = 32
COUT = 64
KS = 4
EXTENT = 0.08
NCORES = 8
NBLK = N // 8  # 3750 eight-query blocks

_COMPILED = {}


# ----------------------------------------------------------------------------
# Host planning
# ----------------------------------------------------------------------------
def _plan(qry_idx):
    deg = np.bincount(qry_idx, minlength=N)
    bsz = deg.reshape(NBLK, 8).sum(1)
    bstart = np.concatenate([[0], np.cumsum(bsz)]).astype(np.int64)
    per = [NBLK // NCORES + (1 if c < NBLK % NCORES else 0) for c in range(NCORES)]
    b0 = np.concatenate([[0], np.cumsum(per)]).astype(np.int64)
    plans = []
    for c in range(NCORES):
        blocks = list(range(b0[c], b0[c + 1]))
        asc = sorted(blocks, key=lambda b: bsz[b])
        chunks = []
        lo, hi = 0, len(asc) - 1
        while lo <= hi:
            if lo == hi:
                chunks.append((asc[hi], None)); break
            if bsz[asc[hi]] + bsz[asc[lo]] <= 128:
                chunks.append((asc[hi], asc[lo])); hi -= 1; lo += 1
            else:
                chunks.append((asc[hi], None)); hi -= 1
        plans.append(dict(blocks=blocks, chunks=chunks, q0=int(8 * b0[c]),
                          nq=int(8 * (b0[c + 1] - b0[c]))))
    return plans, bstart, bsz


def _pack_core(plan_c, bstart, pos, feats_bf, qry_idx, src_idx, NCHP):
    """Per-slot payload: pos coord-major, feats bf16, int16 scatter indices."""
    possrc = np.zeros((128, 4, NCHP), np.float32)
    posqry = np.zeros((128, 4, NCHP), np.float32)
    fsrc = np.zeros((128, NCHP, CIN), BF16)
    qloc = np.full((128, NCHP), -1, np.int32)
    for ci, (bA, bB) in enumerate(plan_c['chunks']):
        s = 0
        for half, b in enumerate((bA, bB)):
            if b is None:
                continue
            e0, e1 = int(bstart[b]), int(bstart[b + 1])
            n = e1 - e0
            sl = slice(s, s + n)
            possrc[sl, 0:3, ci] = pos[src_idx[e0:e1]]
            posqry[sl, 0:3, ci] = pos[qry_idx[e0:e1]]
            fsrc[sl, ci, :] = feats_bf[src_idx[e0:e1]]
            qloc[sl, ci] = (qry_idx[e0:e1] - 8 * b) + 8 * half
            s += n
    # scatter index: within each 4-chunk scatter window,
    # idx[slot, ci, t] = (ci%4)*256 + qloc*16 + t  (or -1 for padding)
    t16 = np.arange(16, dtype=np.int32)
    idx = ((np.arange(NCHP, dtype=np.int32)[None, :] % 4) * 256
           + qloc * 16)[:, :, None] + t16[None, None, :]
    idx = np.where((qloc < 0)[:, :, None], -1, idx).astype(np.int16)
    return possrc, posqry, fsrc, idx, qloc.astype(np.float32)


# ----------------------------------------------------------------------------
# Device kernel
# ----------------------------------------------------------------------------
def _build_bass(NCHP, NQ):
    import concourse.bass as bass
    import concourse.tile as tile
    from concourse import bacc, mybir
    from concourse.bass import AP

    f32 = mybir.dt.float32
    f32r = mybir.dt.float32r
    bf16 = mybir.dt.bfloat16
    i32 = mybir.dt.int32
    i16 = mybir.dt.int16
    ALU = mybir.AluOpType
    ACT = mybir.ActivationFunctionType
    EPS = 1e-12
    F4PI = float(4.0 / np.pi)
    SC = 1.5 * (2.0 / EXTENT)  # grid scale folded with coord normalization

    nc = bacc.Bacc("TRN2", target_bir_lowering=False, debug=False)

    W = NCHP
    NGRP = W // 16
    # uneven slab splits (at group boundaries): a tiny first slab so the
    # scatter/matmul pipeline starts early; later slabs' geometry is fed
    # incrementally into DVE slack during earlier groups
    gb = sorted(set(min(x, NGRP) for x in (0, 8, NGRP)))
    SLABS = [(gb[i] * 16, (gb[i + 1] - gb[i]) * 16)
             for i in range(len(gb) - 1) if gb[i + 1] > gb[i]]
    SLAB_OF = {}
    for si, (o0, Ws) in enumerate(SLABS):
        for g in range(o0 // 16, (o0 + Ws) // 16):
            SLAB_OF[g] = si

    possrc = nc.dram_tensor("possrc", (128, 4, W), f32, kind="ExternalInput")
    posqry = nc.dram_tensor("posqry", (128, 4, W), f32, kind="ExternalInput")
    fsrc = nc.dram_tensor("fsrc", (128, W, CIN), bf16, kind="ExternalInput")
    sidx = nc.dram_tensor("sidx", (128, W, 16), i16, kind="ExternalInput")
    qlocf = nc.dram_tensor("qlocf", (128, W), f32, kind="ExternalInput")
    g2 = nc.dram_tensor("g2", (128, 16 * 64), f32, kind="ExternalInput")
    featsT = nc.dram_tensor("featsT", (CIN, NQ), bf16, kind="ExternalInput")
    denw = nc.dram_tensor("denw", (CIN, COUT), bf16, kind="ExternalInput")
    denb = nc.dram_tensor("denb", (COUT, 1), f32, kind="ExternalInput")

    outconvT = nc.dram_tensor("outconvT", (COUT, NQ), f32, kind="ExternalOutput")
    outdenseT = nc.dram_tensor("outdenseT", (COUT, NQ), f32, kind="ExternalOutput")

    with tile.TileContext(nc) as tc:
        with tc.tile_pool(name="inp", bufs=1) as inp, \
             tc.tile_pool(name="geo", bufs=1) as geo, \
             tc.tile_pool(name="w4p", bufs=2) as w4p, \
             tc.tile_pool(name="tmp", bufs=1) as tmp, \
             tc.tile_pool(name="lp", bufs=3) as lpool, \
             tc.tile_pool(name="rp", bufs=3) as rpool, \
             tc.tile_pool(name="at", bufs=2) as atp, \
             tc.tile_pool(name="outp", bufs=2) as outp, \
             tc.tile_pool(name="ps1", bufs=3, space="PSUM") as ps1, \
             tc.tile_pool(name="ps2", bufs=1, space="PSUM") as ps2, \
             tc.tile_pool(name="ps3", bufs=1, space="PSUM") as ps3:

            # ---------------- input DMAs ----------------
            t_ps = inp.tile([128, 4, W], f32)
            t_pq = inp.tile([128, 4, W], f32)
            t_f = inp.tile([128, W, CIN], bf16)
            t_si = inp.tile([128, W, 16], i16)
            t_ql = inp.tile([128, W], f32)
            t_g2 = inp.tile([128, 16 * 64], f32)
            t_ftT = inp.tile([CIN, NQ], bf16)
            t_dw = inp.tile([CIN, COUT], bf16)
            t_db = inp.tile([COUT, 1], f32)
            # slab-0 pos first so geometry can start early
            W0 = SLABS[0][1]
            nc.sync.dma_start(t_ps[:, :, 0:W0], possrc[:, :, 0:W0])
            nc.sync.dma_start(t_pq[:, :, 0:W0], posqry[:, :, 0:W0])
            nc.sync.dma_start(t_si[:], sidx[:])
            nc.sync.dma_start(t_f[:], fsrc[:])
            nc.sync.dma_start(t_dw[:], denw[:])
            nc.sync.dma_start(t_db[:], denb[:])
            nc.sync.dma_start(t_ps[:, :, W0:W], possrc[:, :, W0:W])
            nc.sync.dma_start(t_pq[:, :, W0:W], posqry[:, :, W0:W])
            nc.sync.dma_start(t_ftT[:], featsT[:])
            nc.sync.dma_start(t_g2[:], g2[:])
            nc.sync.dma_start(t_ql[:], qlocf[:])

            # iota constants: io4m = j - 1.5 (j=0..3), io16 = 0..15
            io4i = tmp.tile([128, 4], i32)
            nc.gpsimd.iota(io4i[:], pattern=[[1, 4]], base=0, channel_multiplier=0)
            io4m = tmp.tile([128, 4], f32)
            nc.scalar.activation(io4m[:], io4i[:], ACT.Copy, bias=-1.5)
            t_g2r = inp.tile([128, 16 * 64], f32r)
            nc.vector.tensor_copy(t_g2r[:], t_g2[:])
            io16i = tmp.tile([128, 16], i32)
            nc.gpsimd.iota(io16i[:], pattern=[[1, 16]], base=0,
                           channel_multiplier=0)
            io16 = tmp.tile([128, 16], f32)
            nc.scalar.activation(io16[:], io16i[:], ACT.Copy)

            TT = nc.vector.tensor_tensor
            TS = nc.vector.tensor_scalar
            STT = nc.vector.scalar_tensor_tensor
            AA = nc.scalar.activation

            def fl(t, n, off=0):  # flat [128, n] view
                return AP(t.tensor, t[:].offset + off, [t[:].ap[0], [1, n]])

            def sl(t, off, n, *dims):  # strided view: dims = (stride, count)*
                pat = [t[:].ap[0]] + [[s, c] for (s, c) in dims] if dims else \
                      [t[:].ap[0], [1, n]]
                return AP(t.tensor, t[:].offset + off, pat)

            # ------------- dense branch, issued one segment per group ------
            def dense_seg(s):
                j0 = s * 512
                j1 = min(NQ, j0 + 512)
                pd = ps3.tile([COUT, 512], f32, space="PSUM", tag="den")
                nc.tensor.matmul(
                    out=pd[:, 0:j1 - j0],
                    lhsT=t_dw[:],
                    rhs=t_ftT[:, j0:j1],
                    start=True, stop=True)
                odt = outp.tile([COUT, 512], f32, tag="odst")
                nc.scalar.activation(odt[:, 0:j1 - j0], pd[:, 0:j1 - j0],
                                     ACT.Identity, bias=t_db[:, 0:1])
                nc.sync.dma_start(outdenseT[:, j0:j1], odt[:, 0:j1 - j0])

            NDSEG = (NQ + 511) // 512

            # ---------------- geometry (per slab) ----------------
            def geometry(o0, Ws):  # generator: yields between segments
                """Compute w4b [128,3,Ws,4] bf16, zy/qoh [128,Ws,16] bf16
                for chunk columns [o0, o0+Ws)."""
                def gt(shape, dt_, tag):
                    return geo.tile(shape, dt_, name=f"{tag}_{o0}", tag=tag)

                dd = gt([128, 3, Ws], f32, "dd")
                sq3 = gt([128, 3, Ws], f32, "sq3")
                # pos tiles are [128, 4, W]; coord-major slab view
                pv = (W, 3), (1, Ws)
                TT(out=fl(dd, 3 * Ws), in0=sl(t_ps, o0, 0, *pv),
                   in1=sl(t_pq, o0, 0, *pv), op=ALU.subtract)
                TT(out=fl(sq3, 3 * Ws), in0=fl(dd, 3 * Ws), in1=fl(dd, 3 * Ws),
                   op=ALU.mult)

                def gW(tag):
                    return gt([128, Ws], f32, tag)

                xy2 = gW("xy2"); sq = gW("sq"); norm = gW("norm")
                nxy = gW("nxy"); azn = gW("azn"); den1 = gW("den1")
                rd1 = gW("rd1"); t1s = gW("t1s"); s1 = gW("s1")
                den2 = gW("den2"); rd2 = gW("rd2"); s2 = gW("s2")
                pole = gt([128, Ws], i32, "pole")
                wq = gW("wq"); zsg = gW("zsg"); zcp = gW("zcp")
                sqxy = gW("sqxy"); nrm = gW("nrm")
                abr = gt([128, Ws], i32, "abr")

                zofs = 2 * Ws
                yield
                TT(out=xy2[:], in0=sl(sq3, 0, Ws), in1=sl(sq3, Ws, Ws),
                   op=ALU.add)
                TT(out=sq[:], in0=xy2[:], in1=sl(sq3, zofs, Ws), op=ALU.add)
                AA(norm[:], sq[:], ACT.Sqrt)
                AA(nxy[:], xy2[:], ACT.Sqrt)
                AA(azn[:], sl(dd, zofs, Ws), ACT.Abs)
                STT(out=den1[:], in0=azn[:], scalar=EPS, in1=norm[:],
                    op0=ALU.add, op1=ALU.add)
                nc.vector.reciprocal_approx_fast(rd1[:], den1[:])
                TT(out=t1s[:], in0=norm[:], in1=rd1[:], op=ALU.mult)
                AA(s1[:], t1s[:], ACT.Sqrt, scale=3.0)
                TS(den2[:], nxy[:], EPS, None, op0=ALU.add)
                nc.vector.reciprocal_approx_fast(rd2[:], den2[:])
                TT(out=s2[:], in0=norm[:], in1=rd2[:], op=ALU.mult)
                yield
                STT(out=pole[:], in0=sl(sq3, zofs, Ws), scalar=1.25,
                    in1=xy2[:], op0=ALU.mult, op1=ALU.is_gt)
                nc.vector.tensor_copy(wq[:], s2[:])
                nc.vector.copy_predicated(wq[:], pole[:], s1[:])

                m3 = gt([128, 3, Ws], f32, "m3")
                AA(zsg[:], sl(dd, zofs, Ws), ACT.Sign)
                TT(out=zcp[:], in0=zsg[:], in1=norm[:], op=ALU.mult)
                TS(sl(m3, zofs, Ws), sl(dd, zofs, Ws), 1.5, None, op0=ALU.mult)
                nc.vector.copy_predicated(sl(m3, zofs, Ws), pole[:], zcp[:])

                yield
                xyc = gt([128, 2, Ws], f32, "xyc")
                TT(out=sl(xyc, 0, 0, (Ws, 2), (1, Ws)),
                   in0=sl(dd, 0, 0, (Ws, 2), (1, Ws)),
                   in1=sl(wq, 0, 0, (0, 2), (1, Ws)), op=ALU.mult)
                xyc2 = gt([128, 2, Ws], f32, "xyc2")
                TT(out=fl(xyc2, 2 * Ws), in0=fl(xyc, 2 * Ws),
                   in1=fl(xyc, 2 * Ws), op=ALU.mult)
                TT(out=sqxy[:], in0=sl(xyc2, 0, Ws), in1=sl(xyc2, Ws, Ws),
                   op=ALU.add)
                AA(nrm[:], sqxy[:], ACT.Sqrt)
                axy = gt([128, 2, Ws], f32, "axy")
                AA(fl(axy, 2 * Ws), fl(xyc, 2 * Ws), ACT.Abs)
                TT(out=abr[:], in0=sl(axy, Ws, Ws), in1=sl(axy, 0, Ws),
                   op=ALU.is_le)
                yield
                myx = gt([128, 2, Ws], f32, "myx")
                TS(fl(myx, 2 * Ws), fl(axy, 2 * Ws), EPS, None, op0=ALU.is_lt)
                sf = gt([128, 2, Ws], f32, "sf")
                TT(out=fl(sf, 2 * Ws), in0=fl(xyc, 2 * Ws), in1=fl(myx, 2 * Ws),
                   op=ALU.add)
                rsf = gt([128, 2, Ws], f32, "rsf")
                nc.vector.reciprocal_approx_fast(fl(rsf, 2 * Ws), fl(sf, 2 * Ws))
                rat = gt([128, 2, Ws], f32, "rat")
                TT(out=sl(rat, 0, Ws), in0=sl(xyc, 0, Ws), in1=sl(rsf, Ws, Ws),
                   op=ALU.mult)
                TT(out=sl(rat, Ws, Ws), in0=sl(xyc, Ws, Ws), in1=sl(rsf, 0, Ws),
                   op=ALU.mult)
                at12 = gt([128, 2, Ws], f32, "at12")
                AA(fl(at12, 2 * Ws), fl(rat, 2 * Ws), ACT.Arctan)
                sg = gt([128, 2, Ws], f32, "sg")
                AA(fl(sg, 2 * Ws), fl(xyc, 2 * Ws), ACT.Sign)
                yield
                tmpab = gt([128, 2, Ws], f32, "tmpab")
                TT(out=sl(tmpab, 0, 0, (Ws, 2), (1, Ws)),
                   in0=sl(sg, 0, 0, (Ws, 2), (1, Ws)),
                   in1=sl(nrm, 0, 0, (0, 2), (1, Ws)), op=ALU.mult)
                prod = gt([128, 2, Ws], f32, "prod")
                STT(out=sl(prod, 0, Ws), in0=sl(at12, 0, Ws), scalar=F4PI,
                    in1=sl(tmpab, Ws, Ws), op0=ALU.mult, op1=ALU.mult)
                STT(out=sl(prod, Ws, Ws), in0=sl(at12, Ws, Ws), scalar=F4PI,
                    in1=sl(tmpab, 0, Ws), op0=ALU.mult, op1=ALU.mult)
                nc.vector.tensor_copy(sl(m3, 0, Ws), sl(prod, 0, Ws))
                nc.vector.copy_predicated(sl(m3, 0, Ws), abr[:],
                                          sl(tmpab, 0, Ws))
                nc.vector.tensor_copy(sl(m3, Ws, Ws), sl(tmpab, Ws, Ws))
                nc.vector.copy_predicated(sl(m3, Ws, Ws), abr[:],
                                          sl(prod, Ws, Ws))

                # hat corner weights: w4[j] = relu(1 - |SC*m - (j-1.5)|)
                yield
                d4 = gt([128, 3 * Ws, 4], f32, "d4")
                nd4 = gt([128, 3 * Ws, 4], f32, "nd4")
                w4b = w4p.tile([128, 3, Ws, 4], bf16, name=f"w4b_{o0}", tag="w4b")
                zy = w4p.tile([128, Ws, 16], bf16, name=f"zy_{o0}", tag="zy")

                def hat(coord):
                    co = coord * Ws * 4
                    STT(out=sl(d4, co, 0, (4, Ws), (1, 4)),
                        in0=sl(m3, coord * Ws, 0, (1, Ws), (0, 4)),
                        scalar=SC,
                        in1=sl(io4m, 0, 0, (0, Ws), (1, 4)),
                        op0=ALU.mult, op1=ALU.subtract)
                    AA(sl(nd4, co, 4 * Ws), sl(d4, co, 4 * Ws), ACT.Abs)
                    AA(sl(w4b, co, 4 * Ws), sl(nd4, co, 4 * Ws), ACT.Relu,
                       bias=1.0, scale=-1.0)

                # z and y coords first so zy (the scatter payload) is ready
                # before the x-hat that only R needs
                hat(2)
                hat(1)
                TT(out=zy[:],
                   in0=sl(w4b, 2 * Ws * 4, 0, (4, Ws), (1, 4), (0, 4)),
                   in1=sl(w4b, 1 * Ws * 4, 0, (4, Ws), (0, 4), (1, 4)),
                   op=ALU.mult)
                hat(0)
                qoh = None
                if o0 + Ws == W:  # last slab: one-hot for DVE L fallback
                    qoh = w4p.tile([128, Ws, 16], bf16, name=f"qoh_{o0}",
                                   tag="qoh")
                    TT(out=qoh[:],
                       in0=sl(t_ql, o0, 0, (1, Ws), (0, 16)),
                       in1=sl(io16, 0, 0, (0, Ws), (1, 16)),
                       op=ALU.is_equal)
                slab_data[o0] = (w4b, zy, qoh)

            # ---------------- stage-1 + tap-GEMM (pipelined) -------------
            def tap_gemm(at_t, g):
                po = ps2.tile([COUT, 256], f32, space="PSUM", tag="tap")
                for t in range(16):
                    rhs = AP(at_t.tensor, at_t[:].offset + t,
                             [at_t[:].ap[0], [256, 16], [128, 2], [16, 8]])
                    nc.tensor.matmul(
                        out=po[:],
                        lhsT=t_g2r[:, t * 64:(t + 1) * 64],
                        rhs=rhs,
                        start=(t == 0), stop=(t == 15))
                ost = outp.tile([COUT, 256], f32, tag="ocst")
                nc.scalar.copy(ost[:], po[:])
                nc.sync.dma_start(outconvT[:, g * 256:(g + 1) * 256], ost[:])

            pend = None  # (at_t, g)
            slab_data = {}
            gens = [geometry(*sl_) for sl_ in SLABS]
            for _ in gens[0]:
                pass
            # feed slab k's geometry during the two groups before its slab,
            # spread over the preceding slab's groups
            NSEG = 7  # segments per geometry generator (6 yields + tail)
            feed = {}
            for si in range(1, len(SLABS)):
                start_g = SLABS[si][0] // 16
                prev_g0 = SLABS[si - 1][0] // 16
                span = max(1, start_g - prev_g0)
                for k in range(NSEG):
                    gg = prev_g0 + (k * span) // NSEG
                    feed.setdefault(gg, []).append(si)
            for g in range(NGRP):
                for si_ in feed.get(g, []):
                    next(gens[si_], None)
                si = SLAB_OF[g]
                o0, Ws = SLABS[si]
                if o0 not in slab_data:
                    for _ in gens[si]:
                        pass
                w4b, zy, qoh = slab_data[o0]
                lc = g * 16 - o0  # chunk offset within slab
                # R for 16 chunks in one DVE op: [128, 16, 128] bf16
                R_t = rpool.tile([128, 16, 128], bf16, tag="R")
                TT(out=sl(R_t, 0, 0, (128, 16), (32, 4), (1, 32)),
                   in0=sl(w4b, lc * 4, 0, (4, 16), (1, 4), (0, 32)),
                   in1=sl(t_f, (g * 16) * CIN, 0, (32, 16), (0, 4), (1, 32)),
                   op=ALU.mult)
                # L via gpsimd local_scatter (4 chunks/op); in the back
                # half DVE takes 1-2 quarters (it has slack there, and Pool
                # finishing early hides its slow drain under the tail)
                L_t = lpool.tile([128, 16, 256], bf16, tag="L")
                ndve = 0 if qoh is None else (2 if g == NGRP - 1 else 1)
                for j in range(4 - ndve):
                    c0 = g * 16 + 4 * j
                    nc.gpsimd.local_scatter(
                        out_ap=sl(L_t, j * 1024, 1024),
                        data_ap=sl(zy, (c0 - o0) * 16, 64),
                        idxs_ap=sl(t_si, c0 * 16, 64),
                        channels=128, num_elems=1024, num_idxs=64)
                if ndve:
                    j0_ = 4 - ndve
                    lq = lc + 4 * j0_
                    TT(out=sl(L_t, j0_ * 1024, 0,
                              (256, 4 * ndve), (16, 16), (1, 16)),
                       in0=sl(qoh, lq * 16, 0, (16, 4 * ndve), (1, 16), (0, 16)),
                       in1=sl(zy, lq * 16, 0, (16, 4 * ndve), (0, 16), (1, 16)),
                       op=ALU.mult)
                at_t = atp.tile([128, 16 * 256], f32r, tag="at")
                for j in range(4):
                    ps_t = ps1.tile([128, 1024], f32, space="PSUM", tag="s1")
                    for r in range(4):
                        k = j * 4 + r
                        nc.tensor.matmul(
                            out=ps_t[:, r * 256:(r + 1) * 256],
                            lhsT=R_t[:, k, :], rhs=L_t[:, k, :],
                            start=True, stop=True)
                    dst = at_t[:, j * 1024:(j + 1) * 1024]
                    if j == 3:
                        nc.vector.tensor_copy(dst, ps_t[:])
                    else:
                        nc.scalar.copy(dst, ps_t[:])
                if pend is not None:
                    tap_gemm(*pend)
                pend = (at_t, g)
                if 1 <= g <= NDSEG:
                    dense_seg(g - 1)
            tap_gemm(*pend)
            for s_ in range(min(NGRP - 1, NDSEG), NDSEG):
                dense_seg(s_)

    nc.compile()
    return nc


# ----------------------------------------------------------------------------
# Entry point
# ----------------------------------------------------------------------------
def _prepare(feats, pos, filt, dense_w, dense_b, src_idx, qry_idx):
    feats = np.ascontiguousarray(np.asarray(feats, np.float32))
    pos = np.ascontiguousarray(np.asarray(pos, np.float32))
    filt = np.asarray(filt, np.float32)
    dense_w = np.asarray(dense_w, np.float32)
    dense_b = np.asarray(dense_b, np.float32)
    src_idx = np.asarray(src_idx).astype(np.int64)
    qry_idx = np.asarray(qry_idx).astype(np.int64)

    plans, bstart, bsz = _plan(qry_idx)
    NCH = max(len(p['chunks']) for p in plans)
    NCHP = ((NCH + 15) // 16) * 16
    NQ = NCHP * 16

    feats_bf = feats.astype(BF16)

    # filter regroup: G2[ax*32+c, t*64+o] = filt[az, ay, ax, c, o], t = az*4+ay
    G2 = np.zeros((128, 16 * 64), np.float32)
    for az in range(4):
        for ay in range(4):
            t = az * 4 + ay
            for ax in range(4):
                G2[ax * 32:(ax + 1) * 32, t * 64:(t + 1) * 64] = filt[az, ay, ax]

    in_maps = []
    for c, p in enumerate(plans):
        possrc, posqry, fsrc, sidx, qlocf = _pack_core(
            p, bstart, pos, feats_bf, qry_idx, src_idx, NCHP)
        ftT = np.zeros((CIN, NQ), BF16)
        ftT[:, 0:p['nq']] = feats_bf[p['q0']:p['q0'] + p['nq']].T
        in_maps.append({
            "possrc": possrc, "posqry": posqry, "fsrc": fsrc, "sidx": sidx,
            "qlocf": qlocf,
            "g2": G2, "featsT": ftT, "denw": dense_w.astype(BF16),
            "denb": dense_b.reshape(COUT, 1).astype(np.float32),
        })

    key = (NCHP, NQ)
    if key not in _COMPILED:
        _COMPILED[key] = _build_bass(NCHP, NQ)
    nc = _COMPILED[key]
    return nc, in_maps, plans


def kernel(feats, pos, filt, dense_w, dense_b, src_idx, qry_idx):
    from concourse.bass_utils import run_bass_kernel_spmd

    nc, in_maps, plans = _prepare(feats, pos, filt, dense_w, dense_b,
                                  src_idx, qry_idx)
    res = run_bass_kernel_spmd(nc, in_maps, core_ids=list(range(NCORES)))

    ans_conv = np.zeros((N, COUT), np.float32)
    ans_dense = np.zeros((N, COUT), np.float32)
    for c, p in enumerate(plans):
        outT = res.results[c]["outconvT"]
        for ci, (bA, bB) in enumerate(p['chunks']):
            for half, b in enumerate((bA, bB)):
                if b is None:
                    continue
                cols = ci * 16 + half * 8
                ans_conv[8 * b:8 * b + 8] = outT[:, cols:cols + 8].T
        dT = res.results[c]["outdenseT"]
        ans_dense[p['q0']:p['q0'] + p['nq']] = dT[:, 0:p['nq']].T
    return ans_conv, ans_dense


# revision 43
# speedup vs baseline: 1.0280x; 1.0192x over previous
"""Trainium2 Bass kernel for nn_ContinuousConvolutionBlock (gnn_message_passing).

Strategy (per sharding hint: partition points across 8 cores; each core owns its
queries' scatter-reduce and tap-GEMM; filter + dense weights replicated):

Host side (index plumbing / input marshalling only — zero FLOPs):
  - qry_idx is sorted; queries are grouped into 8-query blocks, blocks paired
    into 128-edge-slot "chunks" (two-pointer bin packing, ~3% padding).
  - Consecutive block ranges are assigned to the 8 cores; per-core per-slot
    payload arrays (pos[src]/pos[qry] coord-major, feats[src] bf16, int16
    scatter indices qloc*16+t, local query ids) are marshalled and DMA'd.

Device side (all FLOP-bearing compute), spread over all five engines:
  - Geometry (DVE+ACT): ball->cube volume-preserving map on UNSCALED deltas
    (map is linear in scale; 2/EXTENT folds into the final grid transform),
    computed in two W-slabs so the group pipeline starts after slab 0;
    4-wide trilinear corner weights via the hat identity
    w4[j] = relu(1 - |g - j|).
  - L build: L[slot, (q16, az, ay)] = one-hot scatter of zy = w4z (x) w4y
    into the query slot — 3 of 4 quarters via gpsimd local_scatter with
    host-precomputed indices (zeroes dst, skips -1 padding), 1 quarter via
    DVE broadcast-mult with a query one-hot (engine balance).
  - R build (DVE): R[slot, (ax,c)] = w4x (x) feats, bf16, 16 chunks/op.
  - Stage-1 (PE): A^T[(ax,c), (q,az,ay)] = R^T @ L per chunk, bf16,
    fp32 PSUM accumulate; PSUM staged to SBUF as bf16 on ACT (3) + DVE (1).
  - Tap-GEMM (PE): out^T += G_t^T @ A^T-slices over 16 taps, bf16, fused
    over group PAIRS (512-wide moving dim) and software-pipelined one pair
    behind stage-1 so the PE stream stays dense.
  - Dense branch (PE, bf16) issued first so PE warms during the prologue.
  Outputs are produced transposed ([64, nq]); host transposes/reorders back.
"""
import sys
import os
sys.path.insert(0, '/opt/trn_rl_repo')
import numpy as np
import ml_dtypes

BF16 = ml_dtypes.bfloat16

N = 30000
CIN = 32
COUT = 64
KS = 4
EXTENT = 0.08
NCORES = 8
NBLK = N // 8  # 3750 eight-query blocks

_COMPILED = {}


# ----------------------------------------------------------------------------
# Host planning
# ----------------------------------------------------------------------------
def _plan(qry_idx):
    deg = np.bincount(qry_idx, minlength=N)
    bsz = deg.reshape(NBLK, 8).sum(1)
    bstart = np.concatenate([[0], np.cumsum(bsz)]).astype(np.int64)
    per = [NBLK // NCORES + (1 if c < NBLK % NCORES else 0) for c in range(NCORES)]
    b0 = np.concatenate([[0], np.cumsum(per)]).astype(np.int64)
    plans = []
    for c in range(NCORES):
        blocks = list(range(b0[c], b0[c + 1]))
        asc = sorted(blocks, key=lambda b: bsz[b])
        chunks = []
        lo, hi = 0, len(asc) - 1
        while lo <= hi:
            if lo == hi:
                chunks.append((asc[hi], None)); break
            if bsz[asc[hi]] + bsz[asc[lo]] <= 128:
                chunks.append((asc[hi], asc[lo])); hi -= 1; lo += 1
            else:
                chunks.append((asc[hi], None)); hi -= 1
        plans.append(dict(blocks=blocks, chunks=chunks, q0=int(8 * b0[c]),
                          nq=int(8 * (b0[c + 1] - b0[c]))))
    return plans, bstart, bsz


def _pack_core(plan_c, bstart, pos, feats_bf, qry_idx, src_idx, NCHP):
    """Per-slot payload: pos coord-major, feats bf16, int16 scatter indices."""
    possrc = np.zeros((128, 4, NCHP), np.float32)
    posqry = np.zeros((128, 4, NCHP), np.float32)
    fsrc = np.zeros((128, NCHP, CIN), BF16)
    qloc = np.full((128, NCHP), -1, np.int32)
    for ci, (bA, bB) in enumerate(plan_c['chunks']):
        s = 0
        for half, b in enumerate((bA, bB)):
            if b is None:
                continue
            e0, e1 = int(bstart[b]), int(bstart[b + 1])
            n = e1 - e0
            sl = slice(s, s + n)
            possrc[sl, 0:3, ci] = pos[src_idx[e0:e1]]
            posqry[sl, 0:3, ci] = pos[qry_idx[e0:e1]]
            fsrc[sl, ci, :] = feats_bf[src_idx[e0:e1]]
            qloc[sl, ci] = (qry_idx[e0:e1] - 8 * b) + 8 * half
            s += n
    # scatter index: within each 4-chunk scatter window,
    # idx[slot, ci, t] = (ci%4)*256 + qloc*16 + t  (or -1 for padding)
    t16 = np.arange(16, dtype=np.int32)
    idx = ((np.arange(NCHP, dtype=np.int32)[None, :] % 4) * 256
           + qloc * 16)[:, :, None] + t16[None, None, :]
    idx = np.where((qloc < 0)[:, :, None], -1, idx).astype(np.int16)
    return possrc, posqry, fsrc, idx, qloc.astype(np.float32)


# ----------------------------------------------------------------------------
# Device kernel
# ----------------------------------------------------------------------------
def _build_bass(NCHP, NQ):
    import concourse.bass as bass
    import concourse.tile as tile
    from concourse import bacc, mybir
    from concourse.bass import AP

    f32 = mybir.dt.float32
    f32r = mybir.dt.float32r
    bf16 = mybir.dt.bfloat16
    i32 = mybir.dt.int32
    i16 = mybir.dt.int16
    ALU = mybir.AluOpType
    ACT = mybir.ActivationFunctionType
    EPS = 1e-12
    F4PI = float(4.0 / np.pi)
    SC = 1.5 * (2.0 / EXTENT)  # grid scale folded with coord normalization

    nc = bacc.Bacc("TRN2", target_bir_lowering=False, debug=False)

    W = NCHP
    NGRP = W // 16
    # uneven slab splits (at group boundaries): a tiny first slab so the
    # scatter/matmul pipeline starts early; later slabs' geometry is fed
    # incrementally into DVE slack during earlier groups
    gb = sorted(set(min(x, NGRP) for x in (0, 8, NGRP)))
    SLABS = [(gb[i] * 16, (gb[i + 1] - gb[i]) * 16)
             for i in range(len(gb) - 1) if gb[i + 1] > gb[i]]
    SLAB_OF = {}
    for si, (o0, Ws) in enumerate(SLABS):
        for g in range(o0 // 16, (o0 + Ws) // 16):
            SLAB_OF[g] = si

    possrc = nc.dram_tensor("possrc", (128, 4, W), f32, kind="ExternalInput")
    posqry = nc.dram_tensor("posqry", (128, 4, W), f32, kind="ExternalInput")
    fsrc = nc.dram_tensor("fsrc", (128, W, CIN), bf16, kind="ExternalInput")
    sidx = nc.dram_tensor("sidx", (128, W, 16), i16, kind="ExternalInput")
    qlocf = nc.dram_tensor("qlocf", (128, W), f32, kind="ExternalInput")
    g2 = nc.dram_tensor("g2", (128, 16 * 64), f32, kind="ExternalInput")
    featsT = nc.dram_tensor("featsT", (CIN, NQ), bf16, kind="ExternalInput")
    denw = nc.dram_tensor("denw", (CIN, COUT), bf16, kind="ExternalInput")
    denb = nc.dram_tensor("denb", (COUT, 1), f32, kind="ExternalInput")

    outconvT = nc.dram_tensor("outconvT", (COUT, NQ), f32, kind="ExternalOutput")
    outdenseT = nc.dram_tensor("outdenseT", (COUT, NQ), f32, kind="ExternalOutput")

    with tile.TileContext(nc) as tc:
        with tc.tile_pool(name="inp", bufs=1) as inp, \
             tc.tile_pool(name="geo", bufs=1) as geo, \
             tc.tile_pool(name="w4p", bufs=2) as w4p, \
             tc.tile_pool(name="tmp", bufs=1) as tmp, \
             tc.tile_pool(name="lp", bufs=3) as lpool, \
             tc.tile_pool(name="rp", bufs=3) as rpool, \
             tc.tile_pool(name="at", bufs=2) as atp, \
             tc.tile_pool(name="outp", bufs=2) as outp, \
             tc.tile_pool(name="ps1", bufs=3, space="PSUM") as ps1, \
             tc.tile_pool(name="ps2", bufs=1, space="PSUM") as ps2, \
             tc.tile_pool(name="ps3", bufs=1, space="PSUM") as ps3:

            # ---------------- input DMAs ----------------
            t_ps = inp.tile([128, 4, W], f32)
            t_pq = inp.tile([128, 4, W], f32)
            t_f = inp.tile([128, W, CIN], bf16)
            t_si = inp.tile([128, W, 16], i16)
            t_ql = inp.tile([128, W], f32)
            t_g2 = inp.tile([128, 16 * 64], f32)
            t_ftT = inp.tile([CIN, NQ], bf16)
            t_dw = inp.tile([CIN, COUT], bf16)
            t_db = inp.tile([COUT, 1], f32)
            # slab-0 pos first so geometry can start early
            W0 = SLABS[0][1]
            nc.sync.dma_start(t_ps[:, :, 0:W0], possrc[:, :, 0:W0])
            nc.sync.dma_start(t_pq[:, :, 0:W0], posqry[:, :, 0:W0])
            nc.sync.dma_start(t_si[:], sidx[:])
            nc.sync.dma_start(t_f[:], fsrc[:])
            nc.sync.dma_start(t_dw[:], denw[:])
            nc.sync.dma_start(t_db[:], denb[:])
            nc.sync.dma_start(t_ps[:, :, W0:W], possrc[:, :, W0:W])
            nc.sync.dma_start(t_pq[:, :, W0:W], posqry[:, :, W0:W])
            nc.sync.dma_start(t_ftT[:], featsT[:])
            nc.sync.dma_start(t_g2[:], g2[:])
            nc.sync.dma_start(t_ql[:], qlocf[:])

            # iota constants: io4m = j - 1.5 (j=0..3), io16 = 0..15
            io4i = tmp.tile([128, 4], i32)
            nc.gpsimd.iota(io4i[:], pattern=[[1, 4]], base=0, channel_multiplier=0)
            io4m = tmp.tile([128, 4], f32)
            nc.scalar.activation(io4m[:], io4i[:], ACT.Copy, bias=-1.5)
            t_g2r = inp.tile([128, 16 * 64], f32r)
            nc.vector.tensor_copy(t_g2r[:], t_g2[:])
            io16i = tmp.tile([128, 16], i32)
            nc.gpsimd.iota(io16i[:], pattern=[[1, 16]], base=0,
                           channel_multiplier=0)
            io16 = tmp.tile([128, 16], f32)
            nc.scalar.activation(io16[:], io16i[:], ACT.Copy)

            TT = nc.vector.tensor_tensor
            TS = nc.vector.tensor_scalar
            STT = nc.vector.scalar_tensor_tensor
            AA = nc.scalar.activation

            def fl(t, n, off=0):  # flat [128, n] view
                return AP(t.tensor, t[:].offset + off, [t[:].ap[0], [1, n]])

            def sl(t, off, n, *dims):  # strided view: dims = (stride, count)*
                pat = [t[:].ap[0]] + [[s, c] for (s, c) in dims] if dims else \
                      [t[:].ap[0], [1, n]]
                return AP(t.tensor, t[:].offset + off, pat)

            # ------------- dense branch, issued one segment per group ------
            def dense_seg(s):
                j0 = s * 512
                j1 = min(NQ, j0 + 512)
                pd = ps3.tile([COUT, 512], f32, space="PSUM", tag="den")
                nc.tensor.matmul(
                    out=pd[:, 0:j1 - j0],
                    lhsT=t_dw[:],
                    rhs=t_ftT[:, j0:j1],
                    start=True, stop=True)
                odt = outp.tile([COUT, 512], f32, tag="odst")
                nc.scalar.activation(odt[:, 0:j1 - j0], pd[:, 0:j1 - j0],
                                     ACT.Identity, bias=t_db[:, 0:1])
                nc.sync.dma_start(outdenseT[:, j0:j1], odt[:, 0:j1 - j0])

            NDSEG = (NQ + 511) // 512

            # ---------------- geometry (per slab) ----------------
            def geometry(o0, Ws):  # generator: yields between segments
                """Compute w4b [128,3,Ws,4] bf16, zy/qoh [128,Ws,16] bf16
                for chunk columns [o0, o0+Ws)."""
                def gt(shape, dt_, tag):
                    return geo.tile(shape, dt_, name=f"{tag}_{o0}", tag=tag)

                dd = gt([128, 3, Ws], f32, "dd")
                sq3 = gt([128, 3, Ws], f32, "sq3")
                # pos tiles are [128, 4, W]; coord-major slab view
                pv = (W, 3), (1, Ws)
                TT(out=fl(dd, 3 * Ws), in0=sl(t_ps, o0, 0, *pv),
                   in1=sl(t_pq, o0, 0, *pv), op=ALU.subtract)
                TT(out=fl(sq3, 3 * Ws), in0=fl(dd, 3 * Ws), in1=fl(dd, 3 * Ws),
                   op=ALU.mult)

                def gW(tag):
                    return gt([128, Ws], f32, tag)

                xy2 = gW("xy2"); sq = gW("sq"); norm = gW("norm")
                nxy = gW("nxy"); azn = gW("azn"); den1 = gW("den1")
                rd1 = gW("rd1"); t1s = gW("t1s"); s1 = gW("s1")
                den2 = gW("den2"); rd2 = gW("rd2"); s2 = gW("s2")
                pole = gt([128, Ws], i32, "pole")
                wq = gW("wq"); zsg = gW("zsg"); zcp = gW("zcp")
                sqxy = gW("sqxy"); nrm = gW("nrm")
                abr = gt([128, Ws], i32, "abr")

                zofs = 2 * Ws
                yield
                TT(out=xy2[:], in0=sl(sq3, 0, Ws), in1=sl(sq3, Ws, Ws),
                   op=ALU.add)
                TT(out=sq[:], in0=xy2[:], in1=sl(sq3, zofs, Ws), op=ALU.add)
                AA(norm[:], sq[:], ACT.Sqrt)
                AA(nxy[:], xy2[:], ACT.Sqrt)
                AA(azn[:], sl(dd, zofs, Ws), ACT.Abs)
                STT(out=den1[:], in0=azn[:], scalar=EPS, in1=norm[:],
                    op0=ALU.add, op1=ALU.add)
                nc.vector.reciprocal_approx_fast(rd1[:], den1[:])
                TT(out=t1s[:], in0=norm[:], in1=rd1[:], op=ALU.mult)
                AA(s1[:], t1s[:], ACT.Sqrt, scale=3.0)
                TS(den2[:], nxy[:], EPS, None, op0=ALU.add)
                nc.vector.reciprocal_approx_fast(rd2[:], den2[:])
                TT(out=s2[:], in0=norm[:], in1=rd2[:], op=ALU.mult)
                yield
                STT(out=pole[:], in0=sl(sq3, zofs, Ws), scalar=1.25,
                    in1=xy2[:], op0=ALU.mult, op1=ALU.is_gt)
                nc.vector.tensor_copy(wq[:], s2[:])
                nc.vector.copy_predicated(wq[:], pole[:], s1[:])

                m3 = gt([128, 3, Ws], f32, "m3")
                AA(zsg[:], sl(dd, zofs, Ws), ACT.Sign)
                TT(out=zcp[:], in0=zsg[:], in1=norm[:], op=ALU.mult)
                TS(sl(m3, zofs, Ws), sl(dd, zofs, Ws), 1.5, None, op0=ALU.mult)
                nc.vector.copy_predicated(sl(m3, zofs, Ws), pole[:], zcp[:])

                yield
                xyc = gt([128, 2, Ws], f32, "xyc")
                TT(out=sl(xyc, 0, 0, (Ws, 2), (1, Ws)),
                   in0=sl(dd, 0, 0, (Ws, 2), (1, Ws)),
                   in1=sl(wq, 0, 0, (0, 2), (1, Ws)), op=ALU.mult)
                xyc2 = gt([128, 2, Ws], f32, "xyc2")
                TT(out=fl(xyc2, 2 * Ws), in0=fl(xyc, 2 * Ws),
                   in1=fl(xyc, 2 * Ws), op=ALU.mult)
                TT(out=sqxy[:], in0=sl(xyc2, 0, Ws), in1=sl(xyc2, Ws, Ws),
                   op=ALU.add)
                AA(nrm[:], sqxy[:], ACT.Sqrt)
                axy = gt([128, 2, Ws], f32, "axy")
                AA(fl(axy, 2 * Ws), fl(xyc, 2 * Ws), ACT.Abs)
                TT(out=abr[:], in0=sl(axy, Ws, Ws), in1=sl(axy, 0, Ws),
                   op=ALU.is_le)
                yield
                myx = gt([128, 2, Ws], f32, "myx")
                TS(fl(myx, 2 * Ws), fl(axy, 2 * Ws), EPS, None, op0=ALU.is_lt)
                sf = gt([128, 2, Ws], f32, "sf")
                TT(out=fl(sf, 2 * Ws), in0=fl(xyc, 2 * Ws), in1=fl(myx, 2 * Ws),
                   op=ALU.add)
                rsf = gt([128, 2, Ws], f32, "rsf")
                nc.vector.reciprocal_approx_fast(fl(rsf, 2 * Ws), fl(sf, 2 * Ws))
                rat = gt([128, 2, Ws], f32, "rat")
                TT(out=sl(rat, 0, Ws), in0=sl(xyc, 0, Ws), in1=sl(rsf, Ws, Ws),
                   op=ALU.mult)
                TT(out=sl(rat, Ws, Ws), in0=sl(xyc, Ws, Ws), in1=sl(rsf, 0, Ws),
                   op=ALU.mult)
                at12 = gt([128, 2, Ws], f32, "at12")
                AA(fl(at12, 2 * Ws), fl(rat, 2 * Ws), ACT.Arctan)
                sg = gt([128, 2, Ws], f32, "sg")
                AA(fl(sg, 2 * Ws), fl(xyc, 2 * Ws), ACT.Sign)
                yield
                tmpab = gt([128, 2, Ws], f32, "tmpab")
                TT(out=sl(tmpab, 0, 0, (Ws, 2), (1, Ws)),
                   in0=sl(sg, 0, 0, (Ws, 2), (1, Ws)),
                   in1=sl(nrm, 0, 0, (0, 2), (1, Ws)), op=ALU.mult)
                prod = gt([128, 2, Ws], f32, "prod")
                STT(out=sl(prod, 0, Ws), in0=sl(at12, 0, Ws), scalar=F4PI,
                    in1=sl(tmpab, Ws, Ws), op0=ALU.mult, op1=ALU.mult)
                STT(out=sl(prod, Ws, Ws), in0=sl(at12, Ws, Ws), scalar=F4PI,
                    in1=sl(tmpab, 0, Ws), op0=ALU.mult, op1=ALU.mult)
                nc.vector.tensor_copy(sl(m3, 0, Ws), sl(prod, 0, Ws))
                nc.vector.copy_predicated(sl(m3, 0, Ws), abr[:],
                                          sl(tmpab, 0, Ws))
                nc.vector.tensor_copy(sl(m3, Ws, Ws), sl(tmpab, Ws, Ws))
                nc.vector.copy_predicated(sl(m3, Ws, Ws), abr[:],
                                          sl(prod, Ws, Ws))

                # hat corner weights: w4[j] = relu(1 - |SC*m - (j-1.5)|)
                yield
                d4 = gt([128, 3 * Ws, 4], f32, "d4")
                nd4 = gt([128, 3 * Ws, 4], f32, "nd4")
                w4b = w4p.tile([128, 3, Ws, 4], bf16, name=f"w4b_{o0}", tag="w4b")
                zy = w4p.tile([128, Ws, 16], bf16, name=f"zy_{o0}", tag="zy")

                def hat(coord):
                    co = coord * Ws * 4
                    STT(out=sl(d4, co, 0, (4, Ws), (1, 4)),
                        in0=sl(m3, coord * Ws, 0, (1, Ws), (0, 4)),
                        scalar=SC,
                        in1=sl(io4m, 0, 0, (0, Ws), (1, 4)),
                        op0=ALU.mult, op1=ALU.subtract)
                    AA(sl(nd4, co, 4 * Ws), sl(d4, co, 4 * Ws), ACT.Abs)
                    AA(sl(w4b, co, 4 * Ws), sl(nd4, co, 4 * Ws), ACT.Relu,
                       bias=1.0, scale=-1.0)

                # z and y coords first so zy (the scatter payload) is ready
                # before the x-hat that only R needs
                hat(2)
                hat(1)
                TT(out=zy[:],
                   in0=sl(w4b, 2 * Ws * 4, 0, (4, Ws), (1, 4), (0, 4)),
                   in1=sl(w4b, 1 * Ws * 4, 0, (4, Ws), (0, 4), (1, 4)),
                   op=ALU.mult)
                hat(0)
                qoh = None
                if o0 + Ws == W:  # last slab: one-hot for DVE L fallback
                    qoh = w4p.tile([128, Ws, 16], bf16, name=f"qoh_{o0}",
                                   tag="qoh")
                    TT(out=qoh[:],
                       in0=sl(t_ql, o0, 0, (1, Ws), (0, 16)),
                       in1=sl(io16, 0, 0, (0, Ws), (1, 16)),
                       op=ALU.is_equal)
                slab_data[o0] = [w4b, zy, qoh]

            # ---------------- stage-1 + tap-GEMM (pipelined) -------------
            def tap_gemm(at_t, g):
                po = ps2.tile([COUT, 256], f32, space="PSUM", tag="tap")
                for t in range(16):
                    rhs = AP(at_t.tensor, at_t[:].offset + t,
                             [at_t[:].ap[0], [256, 16], [128, 2], [16, 8]])
                    nc.tensor.matmul(
                        out=po[:],
                        lhsT=t_g2r[:, t * 64:(t + 1) * 64],
                        rhs=rhs,
                        start=(t == 0), stop=(t == 15))
                ost = outp.tile([COUT, 256], f32, tag="ocst")
                nc.scalar.copy(ost[:], po[:])
                nc.sync.dma_start(outconvT[:, g * 256:(g + 1) * 256], ost[:])

            pend = None  # (at_t, g)
            slab_data = {}
            gens = [geometry(*sl_) for sl_ in SLABS]
            for _ in gens[0]:
                pass
            # feed slab k's geometry during the two groups before its slab,
            # spread over the preceding slab's groups
            NSEG = 7  # segments per geometry generator (6 yields + tail)
            feed = {}
            for si in range(1, len(SLABS)):
                start_g = SLABS[si][0] // 16
                prev_g0 = SLABS[si - 1][0] // 16
                span = max(1, start_g - prev_g0)
                for k in range(NSEG):
                    gg = prev_g0 + (k * span) // NSEG
                    feed.setdefault(gg, []).append(si)
            for g in range(NGRP):
                for si_ in feed.get(g, []):
                    next(gens[si_], None)
                si = SLAB_OF[g]
                o0, Ws = SLABS[si]
                if o0 not in slab_data:
                    for _ in gens[si]:
                        pass
                if g == 1 and len(SLABS) > 1:
                    # lazily build slab-0's query one-hot inside group 1's
                    # Pool-bound window so groups 2-5 can shed a scatter
                    sd0 = slab_data[SLABS[0][0]]
                    if sd0[2] is None:
                        o00, Ws0 = SLABS[0]
                        q0t = w4p.tile([128, Ws0, 16], bf16, name="qoh0",
                                       tag="qoh0")
                        TT(out=q0t[:],
                           in0=sl(t_ql, o00, 0, (1, Ws0), (0, 16)),
                           in1=sl(io16, 0, 0, (0, Ws0), (1, 16)),
                           op=ALU.is_equal)
                        sd0[2] = q0t
                w4b, zy, qoh = slab_data[o0]
                lc = g * 16 - o0  # chunk offset within slab
                # R for 16 chunks in one DVE op: [128, 16, 128] bf16
                R_t = rpool.tile([128, 16, 128], bf16, tag="R")
                TT(out=sl(R_t, 0, 0, (128, 16), (32, 4), (1, 32)),
                   in0=sl(w4b, lc * 4, 0, (4, 16), (1, 4), (0, 32)),
                   in1=sl(t_f, (g * 16) * CIN, 0, (32, 16), (0, 4), (1, 32)),
                   op=ALU.mult)
                # L via gpsimd local_scatter (4 chunks/op); in the back
                # half DVE takes 1-2 quarters (it has slack there, and Pool
                # finishing early hides its slow drain under the tail)
                L_t = lpool.tile([128, 16, 256], bf16, tag="L")
                # DVE takes one scatter-quarter where it has slack: groups
                # 2-5 (feed groups 6-7 and the final group stay all-Pool so
                # the slab-1 feed and the end chain aren't serialized on DVE)
                if qoh is None or g in (0, 1, 6, 7) or g == NGRP - 1:
                    ndve = 0
                else:
                    ndve = 1
                act_all = 2 <= g <= 5  # ACT takes all 4 PSUM copies here
                for j in range(4 - ndve):
                    c0 = g * 16 + 4 * j
                    nc.gpsimd.local_scatter(
                        out_ap=sl(L_t, j * 1024, 1024),
                        data_ap=sl(zy, (c0 - o0) * 16, 64),
                        idxs_ap=sl(t_si, c0 * 16, 64),
                        channels=128, num_elems=1024, num_idxs=64)
                if ndve:
                    j0_ = 4 - ndve
                    lq = lc + 4 * j0_
                    TT(out=sl(L_t, j0_ * 1024, 0,
                              (256, 4 * ndve), (16, 16), (1, 16)),
                       in0=sl(qoh, lq * 16, 0, (16, 4 * ndve), (1, 16), (0, 16)),
                       in1=sl(zy, lq * 16, 0, (16, 4 * ndve), (0, 16), (1, 16)),
                       op=ALU.mult)
                at_t = atp.tile([128, 16 * 256], f32r, tag="at")
                for j in range(4):
                    ps_t = ps1.tile([128, 1024], f32, space="PSUM", tag="s1")
                    for r in range(4):
                        k = j * 4 + r
                        nc.tensor.matmul(
                            out=ps_t[:, r * 256:(r + 1) * 256],
                            lhsT=R_t[:, k, :], rhs=L_t[:, k, :],
                            start=True, stop=True)
                    dst = at_t[:, j * 1024:(j + 1) * 1024]
                    if j == 3 and not act_all:
                        nc.vector.tensor_copy(dst, ps_t[:])
                    else:
                        nc.scalar.copy(dst, ps_t[:])
                if pend is not None:
                    tap_gemm(*pend)
                pend = (at_t, g)
                if 1 <= g <= NDSEG:
                    dense_seg(g - 1)
            tap_gemm(*pend)
            for s_ in range(min(NGRP - 1, NDSEG), NDSEG):
                dense_seg(s_)

    nc.compile()
    return nc


# ----------------------------------------------------------------------------
# Entry point
# ----------------------------------------------------------------------------
def _prepare(feats, pos, filt, dense_w, dense_b, src_idx, qry_idx):
    feats = np.ascontiguousarray(np.asarray(feats, np.float32))
    pos = np.ascontiguousarray(np.asarray(pos, np.float32))
    filt = np.asarray(filt, np.float32)
    dense_w = np.asarray(dense_w, np.float32)
    dense_b = np.asarray(dense_b, np.float32)
    src_idx = np.asarray(src_idx).astype(np.int64)
    qry_idx = np.asarray(qry_idx).astype(np.int64)

    plans, bstart, bsz = _plan(qry_idx)
    NCH = max(len(p['chunks']) for p in plans)
    NCHP = ((NCH + 15) // 16) * 16
    NQ = NCHP * 16

    feats_bf = feats.astype(BF16)

    # filter regroup: G2[ax*32+c, t*64+o] = filt[az, ay, ax, c, o], t = az*4+ay
    G2 = np.zeros((128, 16 * 64), np.float32)
    for az in range(4):
        for ay in range(4):
            t = az * 4 + ay
            for ax in range(4):
                G2[ax * 32:(ax + 1) * 32, t * 64:(t + 1) * 64] = filt[az, ay, ax]

    in_maps = []
    for c, p in enumerate(plans):
        possrc, posqry, fsrc, sidx, qlocf = _pack_core(
            p, bstart, pos, feats_bf, qry_idx, src_idx, NCHP)
        ftT = np.zeros((CIN, NQ), BF16)
        ftT[:, 0:p['nq']] = feats_bf[p['q0']:p['q0'] + p['nq']].T
        in_maps.append({
            "possrc": possrc, "posqry": posqry, "fsrc": fsrc, "sidx": sidx,
            "qlocf": qlocf,
            "g2": G2, "featsT": ftT, "denw": dense_w.astype(BF16),
            "denb": dense_b.reshape(COUT, 1).astype(np.float32),
        })

    key = (NCHP, NQ)
    if key not in _COMPILED:
        _COMPILED[key] = _build_bass(NCHP, NQ)
    nc = _COMPILED[key]
    return nc, in_maps, plans


def kernel(feats, pos, filt, dense_w, dense_b, src_idx, qry_idx):
    from concourse.bass_utils import run_bass_kernel_spmd

    nc, in_maps, plans = _prepare(feats, pos, filt, dense_w, dense_b,
                                  src_idx, qry_idx)
    res = run_bass_kernel_spmd(nc, in_maps, core_ids=list(range(NCORES)))

    ans_conv = np.zeros((N, COUT), np.float32)
    ans_dense = np.zeros((N, COUT), np.float32)
    for c, p in enumerate(plans):
        outT = res.results[c]["outconvT"]
        for ci, (bA, bB) in enumerate(p['chunks']):
            for half, b in enumerate((bA, bB)):
                if b is None:
                    continue
                cols = ci * 16 + half * 8
                ans_conv[8 * b:8 * b + 8] = outT[:, cols:cols + 8].T
        dT = res.results[c]["outdenseT"]
        ans_dense[p['q0']:p['q0'] + p['nq']] = dT[:, 0:p['nq']].T
    return ans_conv, ans_dense
